# revision 10
# baseline (speedup 1.0000x reference)
"""CrossDomainAttention TRN2 kernel: 8-core data-parallel over batch.

Reference computation (per batch element, a/b are (L, C) slices):
  ap = a.T (C, L);  q = ap@Wq.T+bq; k,v from b.T
  attn = softmax(q @ k.T / sqrt(L)) (C, C)
  out = LN(attn @ v + ap) over L, returned as the raw (C*L) buffer viewed (L, C)

Fast path (no qkv bias, no gamma/beta — the shipped inputs):
  scores = ap (Wq^T Wk) bp^T, so the q-projection disappears:
    GT = Wk^T Wq (precomputed once, fp8, x64)
    uT[l, d] = (GT^T b_raw)[l, d]  (fp8, keeps the x64)
    scoresT[d, c] = sum_l uT[l, d] a_raw[l, c]   (fp8 DoubleRow matmuls)
  PT[d, c] = exp(scoresT / (sqrt(L)*64)) stored fp8
  v8[d, m] = 16 * (b W_v^T)  fp8;  po = PT^T @ v8 (PSUM, fp8 DoubleRow)
  LayerNorm is scale-invariant per row, so skip the softmax division:
    x = po + (16*rowsum) * ap ;  out = (x - mean(x)) * rsqrt(var(x))
  rowsum via PE matmuls against a column of 16.0s.
"""

import numpy as np

B, L, C = 16, 512, 2048
NCORE = 8
NB = B // NCORE          # batch elements per core
P = 128
F = 512                  # matmul free-dim tile
NLC = L // P             # 4  l/m chunks
NDB = C // P             # 16 d-blocks / c-blocks
NCCH = C // F            # 4  c chunks
LN_EPS = 1e-5
SG = 64.0                # fp8 scale on GT
SV = 16.0                # fp8 scale on v (and the rowsum ones column)
ESC = 1.0 / (float(np.sqrt(L)) * SG)
INV_SQRT_L = 1.0 / float(np.sqrt(L))

_CACHE = {}


def _build_fast(repeat: int = 1):
    import concourse.bass as bass
    import concourse.tile as tile
    from concourse import bacc, mybir
    from concourse.bass import ts, ds
    from concourse.masks import make_identity
    from contextlib import ExitStack

    f32 = mybir.dt.float32
    f32r = mybir.dt.float32r
    f16 = mybir.dt.float16
    bf16 = mybir.dt.bfloat16
    f8 = mybir.dt.float8e4
    AF = mybir.ActivationFunctionType
    ALU = mybir.AluOpType
    DR = mybir.MatmulPerfMode.DoubleRow

    nc = bacc.Bacc("TRN2", target_bir_lowering=False, debug=False,
                   enable_asserts=False)

    a_d = nc.dram_tensor("a", (NB, L, C), f32, kind="ExternalInput").ap()
    b_d = nc.dram_tensor("b", (NB, L, C), f32, kind="ExternalInput").ap()
    w_d = {n: nc.dram_tensor(n, (L, L), f32, kind="ExternalInput").ap()
           for n in ("Wq", "Wk", "Wv")}
    for n in ("bq", "bk", "bv"):
        nc.dram_tensor(n, (L,), f32, kind="ExternalInput")
    nc.dram_tensor("gamma", (L,), f32, kind="ExternalInput")
    nc.dram_tensor("beta", (L,), f32, kind="ExternalInput")
    out_d = nc.dram_tensor("out", (NB, C, L), f32, kind="ExternalOutput").ap()

    with tile.TileContext(nc) as tc, ExitStack() as ctx:
        const = ctx.enter_context(tc.tile_pool(name="const", bufs=1))
        inp = ctx.enter_context(tc.tile_pool(name="inp", bufs=1))
        acts = ctx.enter_context(tc.tile_pool(name="acts", bufs=1))
        small = ctx.enter_context(tc.tile_pool(name="small", bufs=3))
        outp = ctx.enter_context(tc.tile_pool(name="outp", bufs=3))
        ps_mm = ctx.enter_context(tc.tile_pool(name="ps_mm", bufs=2, space="PSUM"))
        ps_out = ctx.enter_context(tc.tile_pool(name="ps_out", bufs=2, space="PSUM"))
        ps_tr = ctx.enter_context(tc.tile_pool(name="ps_tr", bufs=1, space="PSUM"))
        ps_rs = ctx.enter_context(tc.tile_pool(name="ps_rs", bufs=1, space="PSUM"))

        # ---- constants ----
        ident = const.tile([P, P], f32, tag="ident")
        make_identity(nc, ident)
        onesf = const.tile([P, 2], f32, tag="onesf")
        nc.vector.memset(onesf[:], SV)
        ones8 = const.tile([P, 2, 1], f8, tag="ones8")
        nc.vector.tensor_copy(ones8[:, :, 0], onesf[:])
        epsc = const.tile([P, 1], f32, tag="epsc")
        nc.vector.memset(epsc[:], LN_EPS)
        identb = const.tile([P, P], bf16, tag="identb")
        nc.vector.tensor_copy(identb[:], ident[:])

        # ---- weights ----
        # GT8 = SG * (Wk^T Wq): contraction over m using raw [m, l] layouts.
        wk_ld = inp.tile([P, NLC, L], f32, tag="wld")
        nc.sync.dma_start(wk_ld[:], w_d["Wk"].rearrange("(o p) l -> p o l", p=P))
        wq_ld = inp.tile([P, NLC, L], f32, tag="wld2")
        nc.sync.dma_start(wq_ld[:], w_d["Wq"].rearrange("(o p) l -> p o l", p=P))
        wk_bf = inp.tile([P, NLC, L], bf16, tag="wbf")
        nc.vector.tensor_copy(wk_bf[:], wk_ld[:])
        wq_bf = inp.tile([P, NLC, L], bf16, tag="wbf2")
        nc.vector.tensor_copy(wq_bf[:], wq_ld[:])
        GT8 = const.tile([P, NLC, L], f8, tag="GT8")
        for lb in range(NLC):
            ps = ps_out.tile([P, F], f32, tag="out")
            for mi in range(NLC):
                nc.tensor.matmul(ps[:], lhsT=wk_bf[:, mi, ts(lb, P)],
                                 rhs=wq_bf[:, mi, :],
                                 start=(mi == 0), stop=(mi == NLC - 1))
            nc.scalar.mul(GT8[:, lb, :], ps[:], SG)
        # WvT8 = SV * Wv^T (PE transpose of raw Wv)
        wv_ld = inp.tile([P, NLC, L], f32, tag="wld")
        nc.sync.dma_start(wv_ld[:], w_d["Wv"].rearrange("(o p) l -> p o l", p=P))
        WvT8 = const.tile([P, NLC, L], f8, tag="WvT8")
        for li in range(NLC):
            pst = ps_tr.tile([P, F], f32, tag="tr")
            for mi in range(NLC):
                nc.tensor.transpose(pst[:, ts(mi, P)],
                                    wv_ld[:, mi, ts(li, P)], ident[:])
            nc.scalar.mul(WvT8[:, li, :], pst[:], SV)

        # ---- per batch element ----
        for idx, bi in enumerate([i % NB for i in range(NB * repeat)]):
            # loads + fp8 casts, spread across engines so the PE isn't
            # gated on one slow GpSimd cast chain at startup
            a_sb = inp.tile([P, NLC, C], f32, tag="a_sb")
            b_sb = inp.tile([P, NLC, C], f32, tag="b_sb")
            a8 = acts.tile([P, NLC, C], f8, tag="a8", bufs=2)
            b8 = acts.tile([P, NLC, C], f8, tag="b8", bufs=2)
            if idx == 0:
                cast_eng = (nc.vector, nc.scalar, nc.vector, nc.scalar)
            else:
                cast_eng = (nc.vector, nc.scalar, nc.gpsimd, nc.gpsimd)

            def cast(eng, dst, src):
                if eng is nc.scalar:
                    eng.copy(dst, src)
                else:
                    eng.tensor_copy(dst, src)

            for li in range(NLC):
                nc.sync.dma_start(a_sb[:, li, :], a_d[bi, ds(li * P, P), :])
                cast(cast_eng[li], a8[:, li, :], a_sb[:, li, :])
            for li in range(NLC):
                nc.sync.dma_start(b_sb[:, li, :], b_d[bi, ds(li * P, P), :])
                cast(cast_eng[li], b8[:, li, :], b_sb[:, li, :])

            # apT[c_p, gb, m] = a[m, c] transposed (residual, f16).
            # elem 0: fp32 transposes fill otherwise-idle PE at startup;
            # later elems: transpose a bf16 copy at half the PE cost.
            apT = acts.tile([P, NDB, L], f16, tag="apT")
            if idx == 0:
                tsrc, tid = a_sb, ident
            else:
                a_bf = acts.tile([P, NLC, C], bf16, tag="a_bf")
                for li in range(NLC):
                    nc.gpsimd.tensor_copy(a_bf[:, li, :], a_sb[:, li, :])
                tsrc, tid = a_bf, identb
            for gb in range(NDB):
                pst = ps_tr.tile([P, F], f32 if idx == 0 else bf16, tag="tr")
                for li in range(NLC):
                    nc.tensor.transpose(pst[:, ts(li, P)],
                                        tsrc[:, li, ts(gb, P)], tid[:])
                nc.vector.tensor_copy(apT[:, gb, :], pst[:])

            # uT[l_p, lb, d] = (GT^T b)[l, d], fp8 (keeps x SG)
            uT8 = acts.tile([P, NLC, C], f8, tag="uT8")
            for lb in range(NLC):
                for dc2 in range(0, NCCH, 2):
                    ps = ps_mm.tile([P, 2, F], f32, tag="mm")
                    for h in range(2):
                        for kp in range(0, NLC, 2):
                            nc.tensor.matmul(
                                ps[:, h, :],
                                lhsT=GT8[:, kp:kp + 2, ts(lb, P)],
                                rhs=b8[:, kp:kp + 2, ts(dc2 + h, F)],
                                start=(kp == 0), stop=(kp == NLC - 2),
                                perf_mode=DR)
                    nc.scalar.copy(uT8[:, lb, ds(dc2 * F, 2 * F)],
                                   ps.rearrange("p two f -> p (two f)"))

            # v8[d_p, di, m] = SV * (b Wv^T), fp8
            v8 = acts.tile([P, NDB, L], f8, tag="v8")
            for di2 in range(0, NDB, 2):
                ps = ps_mm.tile([P, 2, F], f32, tag="mm")
                for h in range(2):
                    for kp in range(0, NLC, 2):
                        nc.tensor.matmul(
                            ps[:, h, :],
                            lhsT=b8[:, kp:kp + 2, ts(di2 + h, P)],
                            rhs=WvT8[:, kp:kp + 2, :],
                            start=(kp == 0), stop=(kp == NLC - 2),
                            perf_mode=DR)
                nc.scalar.copy(
                    v8.rearrange("p n m -> p (n m)")[:, ds(di2 * F, 2 * F)],
                    ps.rearrange("p two f -> p (two f)"))

            # ---- attention, c-chunk at a time (PV/LN pipelined one behind) ----
            def emit_scores(ci):
                PT8 = acts.tile([P, NDB, F], f8, tag="PT", bufs=2)
                psr4 = small.tile([P, NCCH], f32, tag="psr", bufs=2)
                for di2 in range(0, NDB, 2):
                    ps = ps_mm.tile([P, 2, F], f32, tag="mm")
                    for h in range(2):
                        for kp in range(0, NLC, 2):
                            nc.tensor.matmul(
                                ps[:, h, :],
                                lhsT=uT8[:, kp:kp + 2, ts(di2 + h, P)],
                                rhs=a8[:, kp:kp + 2, ts(ci, F)],
                                start=(kp == 0), stop=(kp == NLC - 2),
                                perf_mode=DR)
                    nc.scalar.activation(
                        PT8.rearrange("p n f -> p (n f)")[:, ds(di2 * F, 2 * F)],
                        ps.rearrange("p two f -> p (two f)"), AF.Exp, scale=ESC)
                # psr[c] = SV * rowsum, per 128-query block
                for cb in range(NCCH):
                    psp = ps_rs.tile([P, 1], f32, tag="rs")
                    for di2 in range(0, NDB, 2):
                        nc.tensor.matmul(psp[:],
                                         lhsT=PT8[:, di2:di2 + 2, ts(cb, P)],
                                         rhs=ones8[:],
                                         start=(di2 == 0), stop=(di2 == NDB - 2),
                                         perf_mode=DR)
                    nc.vector.tensor_copy(psr4[:, cb:cb + 1], psp[:])
                return PT8, psr4

            def emit_pv_ln(ci, PT8, psr4):
                mv4 = small.tile([P, NCCH, 2], f32, tag="mv4", bufs=2)
                xs = []
                for cb in range(NCCH):
                    po = ps_out.tile([P, L], f32, tag="out")
                    for di2 in range(0, NDB, 2):
                        nc.tensor.matmul(po[:],
                                         lhsT=PT8[:, di2:di2 + 2, ts(cb, P)],
                                         rhs=v8[:, di2:di2 + 2, :],
                                         start=(di2 == 0), stop=(di2 == NDB - 2),
                                         perf_mode=DR)
                    x_sb = outp.tile([P, L], f32, tag="x", bufs=6)
                    nc.vector.affine_then_add(x_sb[:], in0=apT[:, ci * NCCH + cb, :],
                                              in1=po[:], scale=psr4[:, cb:cb + 1],
                                              bias=0.0)
                    st6 = small.tile([P, 6], f32, tag="st6")
                    nc.vector.bn_stats(st6[:], x_sb[:])
                    nc.vector.bn_aggr(mv4[:, cb, :], st6[:])
                    xs.append(x_sb)
                sd4 = small.tile([P, 2 * NCCH], f32, tag="sd4")
                nc.scalar.activation(sd4[:, 0:NCCH], mv4[:, :, 1], AF.Sqrt,
                                     bias=epsc[:])
                nc.vector.reciprocal(sd4[:, NCCH:2 * NCCH], sd4[:, 0:NCCH])
                for cb in range(NCCH):
                    gb = ci * NCCH + cb
                    out_sb = outp.tile([P, L], f32, tag="osb", bufs=3)
                    nc.vector.tensor_scalar(out_sb[:], xs[cb][:],
                                            mv4[:, cb, 0:1],
                                            sd4[:, NCCH + cb:NCCH + cb + 1],
                                            ALU.subtract, ALU.mult)
                    nc.sync.dma_start(out_d[bi, ds(gb * P, P), :], out_sb[:])

            prev = None
            for ci in range(NCCH):
                cur = emit_scores(ci)
                if prev is not None:
                    emit_pv_ln(ci - 1, *prev)
                prev = cur
            emit_pv_ln(NCCH - 1, *prev)

    nc.compile()
    return nc


def _build(apply_qkv_bias: bool, apply_gamma_beta: bool, repeat: int = 1):
    if not apply_qkv_bias and not apply_gamma_beta:
        return _build_fast(repeat)
    return _build_generic(apply_qkv_bias, apply_gamma_beta, repeat)


def _build_generic(apply_qkv_bias: bool, apply_gamma_beta: bool, repeat: int = 1):
    import concourse.bass as bass
    import concourse.tile as tile
    from concourse import bacc, mybir
    from concourse.bass import ts, ds
    from concourse.masks import make_identity
    from contextlib import ExitStack

    f32 = mybir.dt.float32
    f16 = mybir.dt.float16
    bf16 = mybir.dt.bfloat16
    AF = mybir.ActivationFunctionType
    ALU = mybir.AluOpType

    nc = bacc.Bacc("TRN2", target_bir_lowering=False, debug=False,
                   enable_asserts=False)

    a_d = nc.dram_tensor("a", (NB, L, C), f32, kind="ExternalInput").ap()
    b_d = nc.dram_tensor("b", (NB, L, C), f32, kind="ExternalInput").ap()
    w_d = {n: nc.dram_tensor(n, (L, L), f32, kind="ExternalInput").ap()
           for n in ("Wq", "Wk", "Wv")}
    bias_d = {n: nc.dram_tensor(n, (L,), f32, kind="ExternalInput").ap()
              for n in ("bq", "bk", "bv")}
    gamma_d = nc.dram_tensor("gamma", (L,), f32, kind="ExternalInput").ap()
    beta_d = nc.dram_tensor("beta", (L,), f32, kind="ExternalInput").ap()
    out_d = nc.dram_tensor("out", (NB, C, L), f32, kind="ExternalOutput").ap()

    def bcast_p(ap1d):
        # broadcast a 1-D DRAM AP across all 128 partitions (DMA source)
        return bass.AP(tensor=ap1d.tensor, offset=ap1d.offset,
                       ap=[[0, P]] + [list(d) for d in ap1d.ap])

    with tile.TileContext(nc) as tc, ExitStack() as ctx:
        const = ctx.enter_context(tc.tile_pool(name="const", bufs=1))
        inp = ctx.enter_context(tc.tile_pool(name="inp", bufs=1))
        acts = ctx.enter_context(tc.tile_pool(name="acts", bufs=1))
        small = ctx.enter_context(tc.tile_pool(name="small", bufs=3))
        outp = ctx.enter_context(tc.tile_pool(name="outp", bufs=3))
        ps_mm = ctx.enter_context(tc.tile_pool(name="ps_mm", bufs=4, space="PSUM"))
        ps_out = ctx.enter_context(tc.tile_pool(name="ps_out", bufs=2, space="PSUM"))
        ps_tr = ctx.enter_context(tc.tile_pool(name="ps_tr", bufs=1, space="PSUM"))
        ps_rs = ctx.enter_context(tc.tile_pool(name="ps_rs", bufs=1, space="PSUM"))

        # ---- constants ----
        ident = const.tile([P, P], f32, tag="ident")
        make_identity(nc, ident)
        cpack = const.tile([P, 16], f32, tag="cpack")
        nc.vector.memset(cpack[:, 0:1], 1.0)      # ones column (reduce rhs)
        nc.vector.memset(cpack[:, 1:2], LN_EPS)
        ones = cpack[:, 0:1]
        eps = cpack[:, 1:2]
        bias_col = {}
        bv_bc = None
        if apply_qkv_bias:
            for i, n in enumerate(("bq", "bk")):
                dst = cpack[:, 2 + 4 * i: 2 + 4 * (i + 1)]
                nc.sync.dma_start(dst, bias_d[n].rearrange("(o p) -> p o", p=P))
                bias_col[n] = dst
            bv_bc = const.tile([P, L], f32, tag="bv_bc")
            nc.sync.dma_start(bv_bc[:], bcast_p(bias_d["bv"]))
        if apply_gamma_beta:
            gb_pack = const.tile([P, 2, L], f32, tag="gb")
            nc.sync.dma_start(gb_pack[:, 0, :], bcast_p(gamma_d))
            nc.sync.dma_start(gb_pack[:, 1, :], bcast_p(beta_d))

        # ---- weights: load W[m, l] fp32, transpose -> WT[l_p, li, m] bf16 ----
        WT = {}
        for n in ("Wq", "Wk", "Wv"):
            wld = inp.tile([P, NLC, F], f32, tag="bh")
            nc.sync.dma_start(wld[:], w_d[n].rearrange("(o p) l -> p o l", p=P))
            wt = const.tile([P, NLC, L], bf16, tag=f"WT_{n}")
            for mi in range(NLC):
                pst = ps_tr.tile([P, F], f32, tag="tr")
                for li in range(NLC):
                    nc.tensor.transpose(pst[:, ts(li, P)],
                                        wld[:, mi, ts(li, P)], ident[:])
                nc.vector.tensor_copy(
                    wt[:, :, ts(mi, P)],
                    pst.rearrange("p (li f) -> p li f", f=P))
            WT[n] = wt

        # ---- per batch element ----
        for bi in [i % NB for i in range(NB * repeat)]:
            # a: load fp32 per l-chunk, cast to bf16 (GPSIMD)
            a_sb = inp.tile([P, NLC, C], f32, tag="a")
            a_bf = acts.tile([P, NLC, C], bf16, tag="a_bf")
            for li in range(NLC):
                nc.sync.dma_start(a_sb[:, li, :],
                                  a_d[bi, ds(li * P, P), :])
                nc.gpsimd.tensor_copy(a_bf[:, li, :], a_sb[:, li, :])
            apT = acts.tile([P, NDB, L], f16, tag="apT")

            def emit_apt(gb, a_sb=a_sb, apT=apT):
                pst = ps_tr.tile([P, F], f32, tag="tr", name="pst")
                for li in range(NLC):
                    nc.tensor.transpose(pst[:, ts(li, P)],
                                        a_sb[:, li, ts(gb, P)], ident[:])
                nc.vector.tensor_copy(apT[:, gb, :], pst[:])

            # a few up-front (they only need a_sb); the rest interleave into
            # the QKV loop as PE filler between PSUM-limited matmul groups
            apt_queue = list(range(NDB))
            for _ in range(4):
                emit_apt(apt_queue.pop(0))

            # b: load fp32 in halves, cast to bf16
            b_bf = acts.tile([P, NLC, C], bf16, tag="bpt", bufs=2)
            for h in range(2):
                b_sb = inp.tile([P, 2, C], f32, tag="bh")
                nc.sync.dma_start(
                    b_sb[:],
                    b_d[bi, ds(h * 2 * P, 2 * P), :].rearrange(
                        "(o p) c -> p o c", p=P))
                for li in range(2):
                    nc.gpsimd.tensor_copy(b_bf[:, h * 2 + li, :], b_sb[:, li, :])

            # qT[m_p, mi, c], kT[m_p, mi, c] (bf16)
            qT = acts.tile([P, NLC, C], bf16, tag="qT")
            kT = acts.tile([P, NLC, C], bf16, tag="kT")
            for wname, bname, src, dst in (("Wq", "bq", a_bf, qT),
                                           ("Wk", "bk", b_bf, kT)):
                for mi in range(NLC):
                    for ci in range(NCCH):
                        ps = ps_mm.tile([P, F], f32, tag="mm")
                        for li in range(NLC):
                            nc.tensor.matmul(ps[:],
                                             lhsT=WT[wname][:, li, ts(mi, P)],
                                             rhs=src[:, li, ts(ci, F)],
                                             start=(li == 0), stop=(li == NLC - 1))
                        dslice = dst[:, mi, ts(ci, F)]
                        if apply_qkv_bias:
                            nc.scalar.activation(dslice, ps[:], AF.Identity,
                                                 bias=bias_col[bname][:, mi:mi + 1])
                        elif (mi + ci) % 2 == 0:
                            nc.scalar.copy(dslice, ps[:])
                        else:
                            nc.vector.tensor_copy(dslice, ps[:])
                        if apt_queue:
                            emit_apt(apt_queue.pop(0))

            # v[d_p, di, m] (bf16)
            v_sb = acts.tile([P, NDB, L], bf16, tag="v")
            for di in range(NDB):
                ps = ps_mm.tile([P, F], f32, tag="mm")
                for li in range(NLC):
                    nc.tensor.matmul(ps[:], lhsT=b_bf[:, li, ts(di, P)],
                                     rhs=WT["Wv"][:, li, :],
                                     start=(li == 0), stop=(li == NLC - 1))
                if di % 2 == 0:
                    nc.scalar.copy(v_sb[:, di, :], ps[:])
                else:
                    nc.vector.tensor_copy(v_sb[:, di, :], ps[:])
                if apply_qkv_bias:
                    nc.vector.tensor_add(v_sb[:, di, :], v_sb[:, di, :],
                                         bv_bc[:, :])

            # ---- attention, c-chunk at a time ----
            for ci in range(NCCH):
                PT = acts.tile([P, NDB, F], bf16, tag="bpt", bufs=2)
                sumacc = small.tile([P, F], f32, tag="sumacc")
                for di in range(NDB):
                    ps = ps_mm.tile([P, F], f32, tag="mm")
                    for mi in range(NLC):
                        nc.tensor.matmul(ps[:], lhsT=kT[:, mi, ts(di, P)],
                                         rhs=qT[:, mi, ts(ci, F)],
                                         start=(mi == 0), stop=(mi == NLC - 1))
                    nc.scalar.activation(PT[:, di, :], ps[:], AF.Exp,
                                         scale=INV_SQRT_L)
                    if di == 0:
                        nc.vector.tensor_copy(sumacc[:], PT[:, di, :])
                    else:
                        nc.vector.tensor_add(sumacc[:], sumacc[:], PT[:, di, :])

                for cb in range(NCCH):
                    gb = ci * NCCH + cb
                    # rowsum over partitions for these 128 queries
                    psr = ps_rs.tile([P, 1], f32, tag="rs")
                    nc.tensor.matmul(psr[:], lhsT=sumacc[:, ts(cb, P)],
                                     rhs=ones, start=True, stop=True)
                    stats = small.tile([P, 16], f32, tag="stats")
                    rs = stats[:, 0:1]
                    nc.vector.reciprocal(rs, psr[:])
                    # PV
                    po = ps_out.tile([P, L], f32, tag="out")
                    for di in range(NDB):
                        nc.tensor.matmul(po[:], lhsT=PT[:, di, ts(cb, P)],
                                         rhs=v_sb[:, di, :],
                                         start=(di == 0), stop=(di == NDB - 1))
                    out_sb = outp.tile([P, L], f32, tag="out")
                    nc.vector.tensor_scalar_mul(out_sb[:], po[:], rs)
                    nc.vector.tensor_add(out_sb[:], out_sb[:], apT[:, gb, :])
                    # LayerNorm over free dim (L)
                    st6 = stats[:, 2:8]
                    mv = stats[:, 8:10]
                    rstd = stats[:, 10:11]
                    nc.vector.bn_stats(st6, out_sb[:])
                    nc.vector.bn_aggr(mv, st6)
                    nc.scalar.activation(rstd, mv[:, 1:2], AF.Sqrt, bias=eps)
                    nc.vector.reciprocal(rstd, rstd)
                    nc.vector.tensor_scalar(out_sb[:], out_sb[:],
                                            mv[:, 0:1], rstd,
                                            ALU.subtract, ALU.mult)
                    if apply_gamma_beta:
                        nc.vector.tensor_mul(out_sb[:], out_sb[:],
                                             gb_pack[:, 0, :])
                        nc.vector.tensor_add(out_sb[:], out_sb[:],
                                             gb_pack[:, 1, :])
                    nc.sync.dma_start(out_d[bi, ds(gb * P, P), :], out_sb[:])

    nc.compile()
    return nc


def _get_nc(apply_qkv_bias, apply_gamma_beta, repeat=1):
    key = (apply_qkv_bias, apply_gamma_beta, repeat)
    if key not in _CACHE:
        _CACHE[key] = _build(*key)
    return _CACHE[key]


def _run(inputs, trace=False):
    from concourse import bass_utils

    a = np.ascontiguousarray(np.asarray(inputs["a"], dtype=np.float32))
    b = np.ascontiguousarray(np.asarray(inputs["b"], dtype=np.float32))
    get = lambda n: np.ascontiguousarray(np.asarray(inputs[n], dtype=np.float32))
    Wq, Wk, Wv = get("Wq"), get("Wk"), get("Wv")
    bq, bk, bv = get("bq"), get("bk"), get("bv")
    gamma, beta = get("gamma"), get("beta")

    apply_qkv_bias = bool(np.any(bq) or np.any(bk) or np.any(bv))
    apply_gamma_beta = bool(np.any(gamma != 1.0) or np.any(beta))
    nc = _get_nc(apply_qkv_bias, apply_gamma_beta)

    in_maps = []
    for c in range(NCORE):
        sl = slice(c * NB, (c + 1) * NB)
        in_maps.append({
            "a": np.ascontiguousarray(a[sl]), "b": np.ascontiguousarray(b[sl]),
            "Wq": Wq, "Wk": Wk, "Wv": Wv,
            "bq": bq, "bk": bk, "bv": bv,
            "gamma": gamma, "beta": beta,
        })
    res = bass_utils.run_bass_kernel_spmd(nc, in_maps,
                                          core_ids=list(range(NCORE)),
                                          trace=trace)
    out = np.concatenate(
        [res.results[c]["out"].reshape(NB, L, C) for c in range(NCORE)], axis=0)
    return out, res


def kernel(**inputs):
    out, _ = _run(inputs, trace=False)
    return out


# revision 13
# speedup vs baseline: 1.0450x; 1.0450x over previous
"""CrossDomainAttention TRN2 kernel: 8-core data-parallel over batch.

Reference computation (per batch element, a/b are (L, C) slices):
  ap = a.T (C, L);  q = ap@Wq.T+bq; k,v from b.T
  attn = softmax(q @ k.T / sqrt(L)) (C, C)
  out = LN(attn @ v + ap) over L, returned as the raw (C*L) buffer viewed (L, C)

Fast path (no qkv bias, no gamma/beta — the shipped inputs):
  scores = ap (Wq^T Wk) bp^T, so the q-projection disappears:
    GT = Wk^T Wq (precomputed once, fp8, x64)
    uT[l, d] = (GT^T b_raw)[l, d]  (fp8, keeps the x64)
    scoresT[d, c] = sum_l uT[l, d] a_raw[l, c]   (fp8 DoubleRow matmuls)
  PT[d, c] = exp(scoresT / (sqrt(L)*64)) stored fp8
  v8[d, m] = 16 * (b W_v^T)  fp8;  po = PT^T @ v8 (PSUM, fp8 DoubleRow)
  LayerNorm is scale-invariant per row, so skip the softmax division:
    x = po + (16*rowsum) * ap ;  out = (x - mean(x)) * rsqrt(var(x))
  rowsum via PE matmuls against a column of 16.0s.
"""

import numpy as np

B, L, C = 16, 512, 2048
NCORE = 8
NB = B // NCORE          # batch elements per core
P = 128
F = 512                  # matmul free-dim tile
NLC = L // P             # 4  l/m chunks
NDB = C // P             # 16 d-blocks / c-blocks
NCCH = C // F            # 4  c chunks
LN_EPS = 1e-5
SG = 64.0                # fp8 scale on GT
SV = 16.0                # fp8 scale on v (and the rowsum ones column)
ESC = 1.0 / (float(np.sqrt(L)) * SG)
INV_SQRT_L = 1.0 / float(np.sqrt(L))

_CACHE = {}


def _build_fast(repeat: int = 1):
    import concourse.bass as bass
    import concourse.tile as tile
    from concourse import bacc, mybir
    from concourse.bass import ts, ds
    from concourse.masks import make_identity
    from contextlib import ExitStack

    f32 = mybir.dt.float32
    f32r = mybir.dt.float32r
    f16 = mybir.dt.float16
    bf16 = mybir.dt.bfloat16
    f8 = mybir.dt.float8e4
    AF = mybir.ActivationFunctionType
    ALU = mybir.AluOpType
    DR = mybir.MatmulPerfMode.DoubleRow

    nc = bacc.Bacc("TRN2", target_bir_lowering=False, debug=False,
                   enable_asserts=False)

    a_d = nc.dram_tensor("a", (NB, L, C), f32, kind="ExternalInput").ap()
    b_d = nc.dram_tensor("b", (NB, L, C), f32, kind="ExternalInput").ap()
    w_d = {n: nc.dram_tensor(n, (L, L), f32, kind="ExternalInput").ap()
           for n in ("Wq", "Wk", "Wv")}
    for n in ("bq", "bk", "bv"):
        nc.dram_tensor(n, (L,), f32, kind="ExternalInput")
    nc.dram_tensor("gamma", (L,), f32, kind="ExternalInput")
    nc.dram_tensor("beta", (L,), f32, kind="ExternalInput")
    out_d = nc.dram_tensor("out", (NB, C, L), f32, kind="ExternalOutput").ap()

    with tile.TileContext(nc) as tc, ExitStack() as ctx:
        const = ctx.enter_context(tc.tile_pool(name="const", bufs=1))
        inp = ctx.enter_context(tc.tile_pool(name="inp", bufs=1))
        acts = ctx.enter_context(tc.tile_pool(name="acts", bufs=1))
        small = ctx.enter_context(tc.tile_pool(name="small", bufs=3))
        outp = ctx.enter_context(tc.tile_pool(name="outp", bufs=3))
        ps_mm = ctx.enter_context(tc.tile_pool(name="ps_mm", bufs=2, space="PSUM"))
        ps_out = ctx.enter_context(tc.tile_pool(name="ps_out", bufs=2, space="PSUM"))
        ps_tr = ctx.enter_context(tc.tile_pool(name="ps_tr", bufs=1, space="PSUM"))
        ps_rs = ctx.enter_context(tc.tile_pool(name="ps_rs", bufs=1, space="PSUM"))

        # ---- constants ----
        ident = const.tile([P, P], f32, tag="ident")
        make_identity(nc, ident)
        onesf = const.tile([P, 2], f32, tag="onesf")
        nc.vector.memset(onesf[:], SV)
        ones8 = const.tile([P, 2, 1], f8, tag="ones8")
        nc.vector.tensor_copy(ones8[:, :, 0], onesf[:])
        epsc = const.tile([P, 1], f32, tag="epsc")
        nc.vector.memset(epsc[:], LN_EPS)
        identb = const.tile([P, P], bf16, tag="identb")
        nc.vector.tensor_copy(identb[:], ident[:])

        # elem-0 input loads first: they gate the PE's first work
        pre_a = inp.tile([P, NLC, C], f32, tag="a_sb")
        pre_b = inp.tile([P, NLC, C], f32, tag="b_sb")
        for li in range(NLC):
            nc.sync.dma_start(pre_a[:, li, :], a_d[0, ds(li * P, P), :])
        for li in range(NLC):
            nc.sync.dma_start(pre_b[:, li, :], b_d[0, ds(li * P, P), :])

        # ---- weights ----
        # GT8 = SG * (Wk^T Wq): contraction over m using raw [m, l] layouts.
        wk_ld = inp.tile([P, NLC, L], f32, tag="wld")
        nc.sync.dma_start(wk_ld[:], w_d["Wk"].rearrange("(o p) l -> p o l", p=P))
        wq_ld = inp.tile([P, NLC, L], f32, tag="wld2")
        nc.sync.dma_start(wq_ld[:], w_d["Wq"].rearrange("(o p) l -> p o l", p=P))
        wk_bf = inp.tile([P, NLC, L], bf16, tag="wbf")
        nc.vector.tensor_copy(wk_bf[:], wk_ld[:])
        wq_bf = inp.tile([P, NLC, L], bf16, tag="wbf2")
        nc.vector.tensor_copy(wq_bf[:], wq_ld[:])
        GT8 = const.tile([P, NLC, L], f8, tag="GT8")
        for lb in range(NLC):
            ps = ps_out.tile([P, F], f32, tag="out")
            for mi in range(NLC):
                nc.tensor.matmul(ps[:], lhsT=wk_bf[:, mi, ts(lb, P)],
                                 rhs=wq_bf[:, mi, :],
                                 start=(mi == 0), stop=(mi == NLC - 1))
            nc.scalar.mul(GT8[:, lb, :], ps[:], SG)
        # WvT8 = SV * Wv^T (PE transpose of raw Wv)
        wv_ld = inp.tile([P, NLC, L], f32, tag="wld")
        nc.sync.dma_start(wv_ld[:], w_d["Wv"].rearrange("(o p) l -> p o l", p=P))
        WvT8 = const.tile([P, NLC, L], f8, tag="WvT8")
        for li in range(NLC):
            pst = ps_tr.tile([P, F], f32, tag="tr")
            for mi in range(NLC):
                nc.tensor.transpose(pst[:, ts(mi, P)],
                                    wv_ld[:, mi, ts(li, P)], ident[:])
            nc.scalar.mul(WvT8[:, li, :], pst[:], SV)

        # ---- per batch element ----
        for idx, bi in enumerate([i % NB for i in range(NB * repeat)]):
            # loads + fp8 casts, spread across engines so the PE isn't
            # gated on one slow GpSimd cast chain at startup
            if idx == 0:
                a_sb, b_sb = pre_a, pre_b
                cast_eng = (nc.vector, nc.scalar, nc.vector, nc.scalar)
            else:
                a_sb = inp.tile([P, NLC, C], f32, tag="a_sb")
                b_sb = inp.tile([P, NLC, C], f32, tag="b_sb")
                cast_eng = (nc.vector, nc.scalar, nc.gpsimd, nc.gpsimd)
            a8 = acts.tile([P, NLC, C], f8, tag="a8", bufs=2)
            b8 = acts.tile([P, NLC, C], f8, tag="b8", bufs=2)

            def cast(eng, dst, src):
                if eng is nc.scalar:
                    eng.copy(dst, src)
                else:
                    eng.tensor_copy(dst, src)

            for li in range(NLC):
                if idx > 0:
                    nc.sync.dma_start(a_sb[:, li, :], a_d[bi, ds(li * P, P), :])
                cast(cast_eng[li], a8[:, li, :], a_sb[:, li, :])
            for li in range(NLC):
                if idx > 0:
                    nc.sync.dma_start(b_sb[:, li, :], b_d[bi, ds(li * P, P), :])
                cast(cast_eng[li], b8[:, li, :], b_sb[:, li, :])

            # apT[c_p, gb, m] = a[m, c] transposed (residual, f16).
            # elem 0: fp32 transposes fill otherwise-idle PE at startup;
            # later elems: transpose a bf16 copy at half the PE cost.
            apT = acts.tile([P, NDB, L], f16, tag="apT")
            if idx == 0:
                tsrc, tid = a_sb, ident
            else:
                a_bf = acts.tile([P, NLC, C], bf16, tag="a_bf")
                for li in range(NLC):
                    cast(cast_eng[li], a_bf[:, li, :], a_sb[:, li, :])
                tsrc, tid = a_bf, identb
            for gb in range(NDB):
                pst = ps_tr.tile([P, F], f32 if idx == 0 else bf16, tag="tr")
                for li in range(NLC):
                    nc.tensor.transpose(pst[:, ts(li, P)],
                                        tsrc[:, li, ts(gb, P)], tid[:])
                nc.vector.tensor_copy(apT[:, gb, :], pst[:])

            # uT[l_p, lb, d] = (GT^T b)[l, d], fp8 (keeps x SG)
            uT8 = acts.tile([P, NLC, C], f8, tag="uT8")
            for lb in range(NLC):
                for dc2 in range(0, NCCH, 2):
                    ps = ps_mm.tile([P, 2, F], f32, tag="mm")
                    for h in range(2):
                        for kp in range(0, NLC, 2):
                            nc.tensor.matmul(
                                ps[:, h, :],
                                lhsT=GT8[:, kp:kp + 2, ts(lb, P)],
                                rhs=b8[:, kp:kp + 2, ts(dc2 + h, F)],
                                start=(kp == 0), stop=(kp == NLC - 2),
                                perf_mode=DR)
                    nc.scalar.copy(uT8[:, lb, ds(dc2 * F, 2 * F)],
                                   ps.rearrange("p two f -> p (two f)"))

            # v8[d_p, di, m] = SV * (b Wv^T), fp8
            v8 = acts.tile([P, NDB, L], f8, tag="v8")
            for di2 in range(0, NDB, 2):
                ps = ps_mm.tile([P, 2, F], f32, tag="mm")
                for h in range(2):
                    for kp in range(0, NLC, 2):
                        nc.tensor.matmul(
                            ps[:, h, :],
                            lhsT=b8[:, kp:kp + 2, ts(di2 + h, P)],
                            rhs=WvT8[:, kp:kp + 2, :],
                            start=(kp == 0), stop=(kp == NLC - 2),
                            perf_mode=DR)
                nc.scalar.copy(
                    v8.rearrange("p n m -> p (n m)")[:, ds(di2 * F, 2 * F)],
                    ps.rearrange("p two f -> p (two f)"))

            # ---- attention, c-chunk at a time (PV/LN pipelined one behind) ----
            def emit_scores(ci):
                PT8 = acts.tile([P, NDB, F], f8, tag="PT", bufs=2)
                psr4 = small.tile([P, NCCH], f32, tag="psr", bufs=2)
                for di2 in range(0, NDB, 2):
                    ps = ps_mm.tile([P, 2, F], f32, tag="mm")
                    for h in range(2):
                        for kp in range(0, NLC, 2):
                            nc.tensor.matmul(
                                ps[:, h, :],
                                lhsT=uT8[:, kp:kp + 2, ts(di2 + h, P)],
                                rhs=a8[:, kp:kp + 2, ts(ci, F)],
                                start=(kp == 0), stop=(kp == NLC - 2),
                                perf_mode=DR)
                    nc.scalar.activation(
                        PT8.rearrange("p n f -> p (n f)")[:, ds(di2 * F, 2 * F)],
                        ps.rearrange("p two f -> p (two f)"), AF.Exp, scale=ESC)
                # psr[c] = SV * rowsum, per 128-query block
                for cb in range(NCCH):
                    psp = ps_rs.tile([P, 1], f32, tag="rs")
                    for di2 in range(0, NDB, 2):
                        nc.tensor.matmul(psp[:],
                                         lhsT=PT8[:, di2:di2 + 2, ts(cb, P)],
                                         rhs=ones8[:],
                                         start=(di2 == 0), stop=(di2 == NDB - 2),
                                         perf_mode=DR)
                    nc.vector.tensor_copy(psr4[:, cb:cb + 1], psp[:])
                return PT8, psr4

            def emit_pv_ln(ci, PT8, psr4):
                mv4 = small.tile([P, NCCH, 2], f32, tag="mv4", bufs=2)
                xs = []
                for cb in range(NCCH):
                    po = ps_out.tile([P, L], f32, tag="out")
                    for di2 in range(0, NDB, 2):
                        nc.tensor.matmul(po[:],
                                         lhsT=PT8[:, di2:di2 + 2, ts(cb, P)],
                                         rhs=v8[:, di2:di2 + 2, :],
                                         start=(di2 == 0), stop=(di2 == NDB - 2),
                                         perf_mode=DR)
                    x_sb = outp.tile([P, L], f32, tag="x", bufs=6)
                    nc.vector.affine_then_add(x_sb[:], in0=apT[:, ci * NCCH + cb, :],
                                              in1=po[:], scale=psr4[:, cb:cb + 1],
                                              bias=0.0)
                    st6 = small.tile([P, 6], f32, tag="st6")
                    nc.vector.bn_stats(st6[:], x_sb[:])
                    nc.vector.bn_aggr(mv4[:, cb, :], st6[:])
                    xs.append(x_sb)
                sd4 = small.tile([P, 2 * NCCH], f32, tag="sd4")
                nc.scalar.activation(sd4[:, 0:NCCH], mv4[:, :, 1], AF.Sqrt,
                                     bias=epsc[:])
                nc.vector.reciprocal(sd4[:, NCCH:2 * NCCH], sd4[:, 0:NCCH])
                for cb in range(NCCH):
                    gb = ci * NCCH + cb
                    out_sb = outp.tile([P, L], f32, tag="osb", bufs=3)
                    nc.vector.tensor_scalar(out_sb[:], xs[cb][:],
                                            mv4[:, cb, 0:1],
                                            sd4[:, NCCH + cb:NCCH + cb + 1],
                                            ALU.subtract, ALU.mult)
                    nc.sync.dma_start(out_d[bi, ds(gb * P, P), :], out_sb[:])

            prev = None
            for ci in range(NCCH):
                cur = emit_scores(ci)
                if prev is not None:
                    emit_pv_ln(ci - 1, *prev)
                prev = cur
            emit_pv_ln(NCCH - 1, *prev)

    nc.compile()
    return nc


def _build(apply_qkv_bias: bool, apply_gamma_beta: bool, repeat: int = 1):
    if not apply_qkv_bias and not apply_gamma_beta:
        return _build_fast(repeat)
    return _build_generic(apply_qkv_bias, apply_gamma_beta, repeat)


def _build_generic(apply_qkv_bias: bool, apply_gamma_beta: bool, repeat: int = 1):
    import concourse.bass as bass
    import concourse.tile as tile
    from concourse import bacc, mybir
    from concourse.bass import ts, ds
    from concourse.masks import make_identity
    from contextlib import ExitStack

    f32 = mybir.dt.float32
    f16 = mybir.dt.float16
    bf16 = mybir.dt.bfloat16
    AF = mybir.ActivationFunctionType
    ALU = mybir.AluOpType

    nc = bacc.Bacc("TRN2", target_bir_lowering=False, debug=False,
                   enable_asserts=False)

    a_d = nc.dram_tensor("a", (NB, L, C), f32, kind="ExternalInput").ap()
    b_d = nc.dram_tensor("b", (NB, L, C), f32, kind="ExternalInput").ap()
    w_d = {n: nc.dram_tensor(n, (L, L), f32, kind="ExternalInput").ap()
           for n in ("Wq", "Wk", "Wv")}
    bias_d = {n: nc.dram_tensor(n, (L,), f32, kind="ExternalInput").ap()
              for n in ("bq", "bk", "bv")}
    gamma_d = nc.dram_tensor("gamma", (L,), f32, kind="ExternalInput").ap()
    beta_d = nc.dram_tensor("beta", (L,), f32, kind="ExternalInput").ap()
    out_d = nc.dram_tensor("out", (NB, C, L), f32, kind="ExternalOutput").ap()

    def bcast_p(ap1d):
        # broadcast a 1-D DRAM AP across all 128 partitions (DMA source)
        return bass.AP(tensor=ap1d.tensor, offset=ap1d.offset,
                       ap=[[0, P]] + [list(d) for d in ap1d.ap])

    with tile.TileContext(nc) as tc, ExitStack() as ctx:
        const = ctx.enter_context(tc.tile_pool(name="const", bufs=1))
        inp = ctx.enter_context(tc.tile_pool(name="inp", bufs=1))
        acts = ctx.enter_context(tc.tile_pool(name="acts", bufs=1))
        small = ctx.enter_context(tc.tile_pool(name="small", bufs=3))
        outp = ctx.enter_context(tc.tile_pool(name="outp", bufs=3))
        ps_mm = ctx.enter_context(tc.tile_pool(name="ps_mm", bufs=4, space="PSUM"))
        ps_out = ctx.enter_context(tc.tile_pool(name="ps_out", bufs=2, space="PSUM"))
        ps_tr = ctx.enter_context(tc.tile_pool(name="ps_tr", bufs=1, space="PSUM"))
        ps_rs = ctx.enter_context(tc.tile_pool(name="ps_rs", bufs=1, space="PSUM"))

        # ---- constants ----
        ident = const.tile([P, P], f32, tag="ident")
        make_identity(nc, ident)
        cpack = const.tile([P, 16], f32, tag="cpack")
        nc.vector.memset(cpack[:, 0:1], 1.0)      # ones column (reduce rhs)
        nc.vector.memset(cpack[:, 1:2], LN_EPS)
        ones = cpack[:, 0:1]
        eps = cpack[:, 1:2]
        bias_col = {}
        bv_bc = None
        if apply_qkv_bias:
            for i, n in enumerate(("bq", "bk")):
                dst = cpack[:, 2 + 4 * i: 2 + 4 * (i + 1)]
                nc.sync.dma_start(dst, bias_d[n].rearrange("(o p) -> p o", p=P))
                bias_col[n] = dst
            bv_bc = const.tile([P, L], f32, tag="bv_bc")
            nc.sync.dma_start(bv_bc[:], bcast_p(bias_d["bv"]))
        if apply_gamma_beta:
            gb_pack = const.tile([P, 2, L], f32, tag="gb")
            nc.sync.dma_start(gb_pack[:, 0, :], bcast_p(gamma_d))
            nc.sync.dma_start(gb_pack[:, 1, :], bcast_p(beta_d))

        # ---- weights: load W[m, l] fp32, transpose -> WT[l_p, li, m] bf16 ----
        WT = {}
        for n in ("Wq", "Wk", "Wv"):
            wld = inp.tile([P, NLC, F], f32, tag="bh")
            nc.sync.dma_start(wld[:], w_d[n].rearrange("(o p) l -> p o l", p=P))
            wt = const.tile([P, NLC, L], bf16, tag=f"WT_{n}")
            for mi in range(NLC):
                pst = ps_tr.tile([P, F], f32, tag="tr")
                for li in range(NLC):
                    nc.tensor.transpose(pst[:, ts(li, P)],
                                        wld[:, mi, ts(li, P)], ident[:])
                nc.vector.tensor_copy(
                    wt[:, :, ts(mi, P)],
                    pst.rearrange("p (li f) -> p li f", f=P))
            WT[n] = wt

        # ---- per batch element ----
        for bi in [i % NB for i in range(NB * repeat)]:
            # a: load fp32 per l-chunk, cast to bf16 (GPSIMD)
            a_sb = inp.tile([P, NLC, C], f32, tag="a")
            a_bf = acts.tile([P, NLC, C], bf16, tag="a_bf")
            for li in range(NLC):
                nc.sync.dma_start(a_sb[:, li, :],
                                  a_d[bi, ds(li * P, P), :])
                nc.gpsimd.tensor_copy(a_bf[:, li, :], a_sb[:, li, :])
            apT = acts.tile([P, NDB, L], f16, tag="apT")

            def emit_apt(gb, a_sb=a_sb, apT=apT):
                pst = ps_tr.tile([P, F], f32, tag="tr", name="pst")
                for li in range(NLC):
                    nc.tensor.transpose(pst[:, ts(li, P)],
                                        a_sb[:, li, ts(gb, P)], ident[:])
                nc.vector.tensor_copy(apT[:, gb, :], pst[:])

            # a few up-front (they only need a_sb); the rest interleave into
            # the QKV loop as PE filler between PSUM-limited matmul groups
            apt_queue = list(range(NDB))
            for _ in range(4):
                emit_apt(apt_queue.pop(0))

            # b: load fp32 in halves, cast to bf16
            b_bf = acts.tile([P, NLC, C], bf16, tag="bpt", bufs=2)
            for h in range(2):
                b_sb = inp.tile([P, 2, C], f32, tag="bh")
                nc.sync.dma_start(
                    b_sb[:],
                    b_d[bi, ds(h * 2 * P, 2 * P), :].rearrange(
                        "(o p) c -> p o c", p=P))
                for li in range(2):
                    nc.gpsimd.tensor_copy(b_bf[:, h * 2 + li, :], b_sb[:, li, :])

            # qT[m_p, mi, c], kT[m_p, mi, c] (bf16)
            qT = acts.tile([P, NLC, C], bf16, tag="qT")
            kT = acts.tile([P, NLC, C], bf16, tag="kT")
            for wname, bname, src, dst in (("Wq", "bq", a_bf, qT),
                                           ("Wk", "bk", b_bf, kT)):
                for mi in range(NLC):
                    for ci in range(NCCH):
                        ps = ps_mm.tile([P, F], f32, tag="mm")
                        for li in range(NLC):
                            nc.tensor.matmul(ps[:],
                                             lhsT=WT[wname][:, li, ts(mi, P)],
                                             rhs=src[:, li, ts(ci, F)],
                                             start=(li == 0), stop=(li == NLC - 1))
                        dslice = dst[:, mi, ts(ci, F)]
                        if apply_qkv_bias:
                            nc.scalar.activation(dslice, ps[:], AF.Identity,
                                                 bias=bias_col[bname][:, mi:mi + 1])
                        elif (mi + ci) % 2 == 0:
                            nc.scalar.copy(dslice, ps[:])
                        else:
                            nc.vector.tensor_copy(dslice, ps[:])
                        if apt_queue:
                            emit_apt(apt_queue.pop(0))

            # v[d_p, di, m] (bf16)
            v_sb = acts.tile([P, NDB, L], bf16, tag="v")
            for di in range(NDB):
                ps = ps_mm.tile([P, F], f32, tag="mm")
                for li in range(NLC):
                    nc.tensor.matmul(ps[:], lhsT=b_bf[:, li, ts(di, P)],
                                     rhs=WT["Wv"][:, li, :],
                                     start=(li == 0), stop=(li == NLC - 1))
                if di % 2 == 0:
                    nc.scalar.copy(v_sb[:, di, :], ps[:])
                else:
                    nc.vector.tensor_copy(v_sb[:, di, :], ps[:])
                if apply_qkv_bias:
                    nc.vector.tensor_add(v_sb[:, di, :], v_sb[:, di, :],
                                         bv_bc[:, :])

            # ---- attention, c-chunk at a time ----
            for ci in range(NCCH):
                PT = acts.tile([P, NDB, F], bf16, tag="bpt", bufs=2)
                sumacc = small.tile([P, F], f32, tag="sumacc")
                for di in range(NDB):
                    ps = ps_mm.tile([P, F], f32, tag="mm")
                    for mi in range(NLC):
                        nc.tensor.matmul(ps[:], lhsT=kT[:, mi, ts(di, P)],
                                         rhs=qT[:, mi, ts(ci, F)],
                                         start=(mi == 0), stop=(mi == NLC - 1))
                    nc.scalar.activation(PT[:, di, :], ps[:], AF.Exp,
                                         scale=INV_SQRT_L)
                    if di == 0:
                        nc.vector.tensor_copy(sumacc[:], PT[:, di, :])
                    else:
                        nc.vector.tensor_add(sumacc[:], sumacc[:], PT[:, di, :])

                for cb in range(NCCH):
                    gb = ci * NCCH + cb
                    # rowsum over partitions for these 128 queries
                    psr = ps_rs.tile([P, 1], f32, tag="rs")
                    nc.tensor.matmul(psr[:], lhsT=sumacc[:, ts(cb, P)],
                                     rhs=ones, start=True, stop=True)
                    stats = small.tile([P, 16], f32, tag="stats")
                    rs = stats[:, 0:1]
                    nc.vector.reciprocal(rs, psr[:])
                    # PV
                    po = ps_out.tile([P, L], f32, tag="out")
                    for di in range(NDB):
                        nc.tensor.matmul(po[:], lhsT=PT[:, di, ts(cb, P)],
                                         rhs=v_sb[:, di, :],
                                         start=(di == 0), stop=(di == NDB - 1))
                    out_sb = outp.tile([P, L], f32, tag="out")
                    nc.vector.tensor_scalar_mul(out_sb[:], po[:], rs)
                    nc.vector.tensor_add(out_sb[:], out_sb[:], apT[:, gb, :])
                    # LayerNorm over free dim (L)
                    st6 = stats[:, 2:8]
                    mv = stats[:, 8:10]
                    rstd = stats[:, 10:11]
                    nc.vector.bn_stats(st6, out_sb[:])
                    nc.vector.bn_aggr(mv, st6)
                    nc.scalar.activation(rstd, mv[:, 1:2], AF.Sqrt, bias=eps)
                    nc.vector.reciprocal(rstd, rstd)
                    nc.vector.tensor_scalar(out_sb[:], out_sb[:],
                                            mv[:, 0:1], rstd,
                                            ALU.subtract, ALU.mult)
                    if apply_gamma_beta:
                        nc.vector.tensor_mul(out_sb[:], out_sb[:],
                                             gb_pack[:, 0, :])
                        nc.vector.tensor_add(out_sb[:], out_sb[:],
                                             gb_pack[:, 1, :])
                    nc.sync.dma_start(out_d[bi, ds(gb * P, P), :], out_sb[:])

    nc.compile()
    return nc


def _get_nc(apply_qkv_bias, apply_gamma_beta, repeat=1):
    key = (apply_qkv_bias, apply_gamma_beta, repeat)
    if key not in _CACHE:
        _CACHE[key] = _build(*key)
    return _CACHE[key]


def _run(inputs, trace=False):
    from concourse import bass_utils

    a = np.ascontiguousarray(np.asarray(inputs["a"], dtype=np.float32))
    b = np.ascontiguousarray(np.asarray(inputs["b"], dtype=np.float32))
    get = lambda n: np.ascontiguousarray(np.asarray(inputs[n], dtype=np.float32))
    Wq, Wk, Wv = get("Wq"), get("Wk"), get("Wv")
    bq, bk, bv = get("bq"), get("bk"), get("bv")
    gamma, beta = get("gamma"), get("beta")

    apply_qkv_bias = bool(np.any(bq) or np.any(bk) or np.any(bv))
    apply_gamma_beta = bool(np.any(gamma != 1.0) or np.any(beta))
    nc = _get_nc(apply_qkv_bias, apply_gamma_beta)

    in_maps = []
    for c in range(NCORE):
        sl = slice(c * NB, (c + 1) * NB)
        in_maps.append({
            "a": np.ascontiguousarray(a[sl]), "b": np.ascontiguousarray(b[sl]),
            "Wq": Wq, "Wk": Wk, "Wv": Wv,
            "bq": bq, "bk": bk, "bv": bv,
            "gamma": gamma, "beta": beta,
        })
    res = bass_utils.run_bass_kernel_spmd(nc, in_maps,
                                          core_ids=list(range(NCORE)),
                                          trace=trace)
    out = np.concatenate(
        [res.results[c]["out"].reshape(NB, L, C) for c in range(NCORE)], axis=0)
    return out, res


def kernel(**inputs):
    out, _ = _run(inputs, trace=False)
    return out


# revision 14
# speedup vs baseline: 1.0774x; 1.0310x over previous
"""CrossDomainAttention TRN2 kernel: 8-core data-parallel over batch.

Reference computation (per batch element, a/b are (L, C) slices):
  ap = a.T (C, L);  q = ap@Wq.T+bq; k,v from b.T
  attn = softmax(q @ k.T / sqrt(L)) (C, C)
  out = LN(attn @ v + ap) over L, returned as the raw (C*L) buffer viewed (L, C)

Fast path (no qkv bias, no gamma/beta — the shipped inputs):
  scores = ap (Wq^T Wk) bp^T, so the q-projection disappears:
    GT = Wk^T Wq (precomputed once, fp8, x64)
    uT[l, d] = (GT^T b_raw)[l, d]  (fp8, keeps the x64)
    scoresT[d, c] = sum_l uT[l, d] a_raw[l, c]   (fp8 DoubleRow matmuls)
  PT[d, c] = exp(scoresT / (sqrt(L)*64)) stored fp8
  v8[d, m] = 16 * (b W_v^T)  fp8;  po = PT^T @ v8 (PSUM, fp8 DoubleRow)
  LayerNorm is scale-invariant per row, so skip the softmax division:
    x = po + (16*rowsum) * ap ;  out = (x - mean(x)) * rsqrt(var(x))
  rowsum via PE matmuls against a column of 16.0s.
"""

import numpy as np

B, L, C = 16, 512, 2048
NCORE = 8
NB = B // NCORE          # batch elements per core
P = 128
F = 512                  # matmul free-dim tile
NLC = L // P             # 4  l/m chunks
NDB = C // P             # 16 d-blocks / c-blocks
NCCH = C // F            # 4  c chunks
LN_EPS = 1e-5
SG = 64.0                # fp8 scale on GT
SV = 16.0                # fp8 scale on v (and the rowsum ones column)
ESC = 1.0 / (float(np.sqrt(L)) * SG)
INV_SQRT_L = 1.0 / float(np.sqrt(L))

_CACHE = {}


def _build_fast(repeat: int = 1):
    import concourse.bass as bass
    import concourse.tile as tile
    from concourse import bacc, mybir
    from concourse.bass import ts, ds
    from concourse.masks import make_identity
    from contextlib import ExitStack

    f32 = mybir.dt.float32
    f32r = mybir.dt.float32r
    f16 = mybir.dt.float16
    bf16 = mybir.dt.bfloat16
    f8 = mybir.dt.float8e4
    AF = mybir.ActivationFunctionType
    ALU = mybir.AluOpType
    DR = mybir.MatmulPerfMode.DoubleRow

    nc = bacc.Bacc("TRN2", target_bir_lowering=False, debug=False,
                   enable_asserts=False)

    a_d = nc.dram_tensor("a", (NB, L, C), f32, kind="ExternalInput").ap()
    b_d = nc.dram_tensor("b", (NB, L, C), f32, kind="ExternalInput").ap()
    w_d = {n: nc.dram_tensor(n, (L, L), f32, kind="ExternalInput").ap()
           for n in ("Wq", "Wk", "Wv")}
    for n in ("bq", "bk", "bv"):
        nc.dram_tensor(n, (L,), f32, kind="ExternalInput")
    nc.dram_tensor("gamma", (L,), f32, kind="ExternalInput")
    nc.dram_tensor("beta", (L,), f32, kind="ExternalInput")
    out_d = nc.dram_tensor("out", (NB, C, L), f32, kind="ExternalOutput").ap()

    with tile.TileContext(nc) as tc, ExitStack() as ctx:
        const = ctx.enter_context(tc.tile_pool(name="const", bufs=1))
        inp = ctx.enter_context(tc.tile_pool(name="inp", bufs=1))
        acts = ctx.enter_context(tc.tile_pool(name="acts", bufs=1))
        small = ctx.enter_context(tc.tile_pool(name="small", bufs=3))
        outp = ctx.enter_context(tc.tile_pool(name="outp", bufs=3))
        ps_mm = ctx.enter_context(tc.tile_pool(name="ps_mm", bufs=2, space="PSUM"))
        ps_out = ctx.enter_context(tc.tile_pool(name="ps_out", bufs=2, space="PSUM"))
        ps_tr = ctx.enter_context(tc.tile_pool(name="ps_tr", bufs=1, space="PSUM"))
        ps_rs = ctx.enter_context(tc.tile_pool(name="ps_rs", bufs=1, space="PSUM"))

        # ---- constants ----
        ident = const.tile([P, P], f32, tag="ident")
        make_identity(nc, ident)
        onesf = const.tile([P, 2], f32, tag="onesf")
        nc.vector.memset(onesf[:], SV)
        ones8 = const.tile([P, 2, 1], f8, tag="ones8")
        nc.vector.tensor_copy(ones8[:, :, 0], onesf[:])
        epsc = const.tile([P, 1], f32, tag="epsc")
        nc.vector.memset(epsc[:], LN_EPS)
        identb = const.tile([P, P], bf16, tag="identb")
        nc.vector.tensor_copy(identb[:], ident[:])

        # elem-0 input loads first: they gate the PE's first work
        pre_a = inp.tile([P, NLC, C], f32, tag="a_sb")
        pre_b = inp.tile([P, NLC, C], f32, tag="b_sb")
        for li in range(NLC):
            nc.sync.dma_start(pre_a[:, li, :], a_d[0, ds(li * P, P), :])
        for li in range(NLC):
            nc.sync.dma_start(pre_b[:, li, :], b_d[0, ds(li * P, P), :])

        # ---- weights ----
        # GT8 = SG * (Wk^T Wq): contraction over m using raw [m, l] layouts.
        wk_ld = inp.tile([P, NLC, L], f32, tag="wld")
        nc.sync.dma_start(wk_ld[:], w_d["Wk"].rearrange("(o p) l -> p o l", p=P))
        wq_ld = inp.tile([P, NLC, L], f32, tag="wld2")
        nc.sync.dma_start(wq_ld[:], w_d["Wq"].rearrange("(o p) l -> p o l", p=P))
        wk_bf = inp.tile([P, NLC, L], bf16, tag="wbf")
        nc.vector.tensor_copy(wk_bf[:], wk_ld[:])
        wq_bf = inp.tile([P, NLC, L], bf16, tag="wbf2")
        nc.vector.tensor_copy(wq_bf[:], wq_ld[:])
        GT8 = const.tile([P, NLC, L], f8, tag="GT8")
        for lb in range(NLC):
            ps = ps_out.tile([P, F], f32, tag="out")
            for mi in range(NLC):
                nc.tensor.matmul(ps[:], lhsT=wk_bf[:, mi, ts(lb, P)],
                                 rhs=wq_bf[:, mi, :],
                                 start=(mi == 0), stop=(mi == NLC - 1))
            nc.scalar.mul(GT8[:, lb, :], ps[:], SG)
        # WvT8 = SV * Wv^T (PE transpose of raw Wv)
        wv_ld = inp.tile([P, NLC, L], f32, tag="wld")
        nc.sync.dma_start(wv_ld[:], w_d["Wv"].rearrange("(o p) l -> p o l", p=P))
        WvT8 = const.tile([P, NLC, L], f8, tag="WvT8")
        for li in range(NLC):
            pst = ps_tr.tile([P, F], f32, tag="tr")
            for mi in range(NLC):
                nc.tensor.transpose(pst[:, ts(mi, P)],
                                    wv_ld[:, mi, ts(li, P)], ident[:])
            nc.scalar.mul(WvT8[:, li, :], pst[:], SV)

        # ---- per batch element ----
        for idx, bi in enumerate([i % NB for i in range(NB * repeat)]):
            # loads + fp8 casts, spread across engines so the PE isn't
            # gated on one slow GpSimd cast chain at startup
            if idx == 0:
                a_sb, b_sb = pre_a, pre_b
                cast_eng = (nc.vector, nc.scalar, nc.vector, nc.scalar)
            else:
                a_sb = inp.tile([P, NLC, C], f32, tag="a_sb")
                b_sb = inp.tile([P, NLC, C], f32, tag="b_sb")
                cast_eng = (nc.vector, nc.scalar, nc.gpsimd, nc.gpsimd)
            a8 = acts.tile([P, NLC, C], f8, tag="a8", bufs=2)
            b8 = acts.tile([P, NLC, C], f8, tag="b8", bufs=2)

            def cast(eng, dst, src):
                if eng is nc.scalar:
                    eng.copy(dst, src)
                else:
                    eng.tensor_copy(dst, src)

            for li in range(NLC):
                if idx > 0:
                    nc.sync.dma_start(a_sb[:, li, :], a_d[bi, ds(li * P, P), :])
                cast(cast_eng[li], a8[:, li, :], a_sb[:, li, :])
            for li in range(NLC):
                if idx > 0:
                    nc.sync.dma_start(b_sb[:, li, :], b_d[bi, ds(li * P, P), :])
                cast(cast_eng[li], b8[:, li, :], b_sb[:, li, :])

            # apT[c_p, gb, m] = a[m, c] transposed (residual, f16).
            # elem 0: fp32 transposes fill otherwise-idle PE at startup;
            # later elems: transpose a bf16 copy at half the PE cost.
            apT = acts.tile([P, NDB, L], f16, tag="apT")
            for gb in range(NDB):
                pst = ps_tr.tile([P, F], f32, tag="tr")
                for li in range(NLC):
                    nc.tensor.transpose(pst[:, ts(li, P)],
                                        a_sb[:, li, ts(gb, P)], ident[:])
                nc.vector.tensor_copy(apT[:, gb, :], pst[:])

            # uT[l_p, lb, d] = (GT^T b)[l, d], fp8 (keeps x SG)
            uT8 = acts.tile([P, NLC, C], f8, tag="uT8")
            for lb in range(NLC):
                for dc2 in range(0, NCCH, 2):
                    ps = ps_mm.tile([P, 2, F], f32, tag="mm")
                    for h in range(2):
                        for kp in range(0, NLC, 2):
                            nc.tensor.matmul(
                                ps[:, h, :],
                                lhsT=GT8[:, kp:kp + 2, ts(lb, P)],
                                rhs=b8[:, kp:kp + 2, ts(dc2 + h, F)],
                                start=(kp == 0), stop=(kp == NLC - 2),
                                perf_mode=DR)
                    nc.scalar.copy(uT8[:, lb, ds(dc2 * F, 2 * F)],
                                   ps.rearrange("p two f -> p (two f)"))

            # v8[d_p, di, m] = SV * (b Wv^T), fp8
            v8 = acts.tile([P, NDB, L], f8, tag="v8")
            for di2 in range(0, NDB, 2):
                ps = ps_mm.tile([P, 2, F], f32, tag="mm")
                for h in range(2):
                    for kp in range(0, NLC, 2):
                        nc.tensor.matmul(
                            ps[:, h, :],
                            lhsT=b8[:, kp:kp + 2, ts(di2 + h, P)],
                            rhs=WvT8[:, kp:kp + 2, :],
                            start=(kp == 0), stop=(kp == NLC - 2),
                            perf_mode=DR)
                nc.scalar.copy(
                    v8.rearrange("p n m -> p (n m)")[:, ds(di2 * F, 2 * F)],
                    ps.rearrange("p two f -> p (two f)"))

            # ---- attention, c-chunk at a time (PV/LN pipelined one behind) ----
            def emit_scores(ci):
                PT8 = acts.tile([P, NDB, F], f8, tag="PT", bufs=2)
                psr4 = small.tile([P, NCCH], f32, tag="psr", bufs=2)
                for di2 in range(0, NDB, 2):
                    ps = ps_mm.tile([P, 2, F], f32, tag="mm")
                    for h in range(2):
                        for kp in range(0, NLC, 2):
                            nc.tensor.matmul(
                                ps[:, h, :],
                                lhsT=uT8[:, kp:kp + 2, ts(di2 + h, P)],
                                rhs=a8[:, kp:kp + 2, ts(ci, F)],
                                start=(kp == 0), stop=(kp == NLC - 2),
                                perf_mode=DR)
                    nc.scalar.activation(
                        PT8.rearrange("p n f -> p (n f)")[:, ds(di2 * F, 2 * F)],
                        ps.rearrange("p two f -> p (two f)"), AF.Exp, scale=ESC)
                # psr[c] = SV * rowsum, per 128-query block
                for cb in range(NCCH):
                    psp = ps_rs.tile([P, 1], f32, tag="rs")
                    for di2 in range(0, NDB, 2):
                        nc.tensor.matmul(psp[:],
                                         lhsT=PT8[:, di2:di2 + 2, ts(cb, P)],
                                         rhs=ones8[:],
                                         start=(di2 == 0), stop=(di2 == NDB - 2),
                                         perf_mode=DR)
                    nc.vector.tensor_copy(psr4[:, cb:cb + 1], psp[:])
                return PT8, psr4

            def emit_pv_ln(ci, PT8, psr4):
                mv4 = small.tile([P, NCCH, 2], f32, tag="mv4", bufs=2)
                xs = []
                for cb in range(NCCH):
                    po = ps_out.tile([P, L], f32, tag="out")
                    for di2 in range(0, NDB, 2):
                        nc.tensor.matmul(po[:],
                                         lhsT=PT8[:, di2:di2 + 2, ts(cb, P)],
                                         rhs=v8[:, di2:di2 + 2, :],
                                         start=(di2 == 0), stop=(di2 == NDB - 2),
                                         perf_mode=DR)
                    x_sb = outp.tile([P, L], f32, tag="x", bufs=6)
                    nc.vector.affine_then_add(x_sb[:], in0=apT[:, ci * NCCH + cb, :],
                                              in1=po[:], scale=psr4[:, cb:cb + 1],
                                              bias=0.0)
                    st6 = small.tile([P, 6], f32, tag="st6")
                    nc.vector.bn_stats(st6[:], x_sb[:])
                    nc.vector.bn_aggr(mv4[:, cb, :], st6[:])
                    xs.append(x_sb)
                sd4 = small.tile([P, 2 * NCCH], f32, tag="sd4")
                nc.scalar.activation(sd4[:, 0:NCCH], mv4[:, :, 1], AF.Sqrt,
                                     bias=epsc[:])
                nc.vector.reciprocal(sd4[:, NCCH:2 * NCCH], sd4[:, 0:NCCH])
                for cb in range(NCCH):
                    gb = ci * NCCH + cb
                    out_sb = outp.tile([P, L], f32, tag="osb", bufs=3)
                    nc.vector.tensor_scalar(out_sb[:], xs[cb][:],
                                            mv4[:, cb, 0:1],
                                            sd4[:, NCCH + cb:NCCH + cb + 1],
                                            ALU.subtract, ALU.mult)
                    nc.sync.dma_start(out_d[bi, ds(gb * P, P), :], out_sb[:])

            prev = None
            for ci in range(NCCH):
                cur = emit_scores(ci)
                if prev is not None:
                    emit_pv_ln(ci - 1, *prev)
                prev = cur
            emit_pv_ln(NCCH - 1, *prev)

    nc.compile()
    return nc


def _build(apply_qkv_bias: bool, apply_gamma_beta: bool, repeat: int = 1):
    if not apply_qkv_bias and not apply_gamma_beta:
        return _build_fast(repeat)
    return _build_generic(apply_qkv_bias, apply_gamma_beta, repeat)


def _build_generic(apply_qkv_bias: bool, apply_gamma_beta: bool, repeat: int = 1):
    import concourse.bass as bass
    import concourse.tile as tile
    from concourse import bacc, mybir
    from concourse.bass import ts, ds
    from concourse.masks import make_identity
    from contextlib import ExitStack

    f32 = mybir.dt.float32
    f16 = mybir.dt.float16
    bf16 = mybir.dt.bfloat16
    AF = mybir.ActivationFunctionType
    ALU = mybir.AluOpType

    nc = bacc.Bacc("TRN2", target_bir_lowering=False, debug=False,
                   enable_asserts=False)

    a_d = nc.dram_tensor("a", (NB, L, C), f32, kind="ExternalInput").ap()
    b_d = nc.dram_tensor("b", (NB, L, C), f32, kind="ExternalInput").ap()
    w_d = {n: nc.dram_tensor(n, (L, L), f32, kind="ExternalInput").ap()
           for n in ("Wq", "Wk", "Wv")}
    bias_d = {n: nc.dram_tensor(n, (L,), f32, kind="ExternalInput").ap()
              for n in ("bq", "bk", "bv")}
    gamma_d = nc.dram_tensor("gamma", (L,), f32, kind="ExternalInput").ap()
    beta_d = nc.dram_tensor("beta", (L,), f32, kind="ExternalInput").ap()
    out_d = nc.dram_tensor("out", (NB, C, L), f32, kind="ExternalOutput").ap()

    def bcast_p(ap1d):
        # broadcast a 1-D DRAM AP across all 128 partitions (DMA source)
        return bass.AP(tensor=ap1d.tensor, offset=ap1d.offset,
                       ap=[[0, P]] + [list(d) for d in ap1d.ap])

    with tile.TileContext(nc) as tc, ExitStack() as ctx:
        const = ctx.enter_context(tc.tile_pool(name="const", bufs=1))
        inp = ctx.enter_context(tc.tile_pool(name="inp", bufs=1))
        acts = ctx.enter_context(tc.tile_pool(name="acts", bufs=1))
        small = ctx.enter_context(tc.tile_pool(name="small", bufs=3))
        outp = ctx.enter_context(tc.tile_pool(name="outp", bufs=3))
        ps_mm = ctx.enter_context(tc.tile_pool(name="ps_mm", bufs=4, space="PSUM"))
        ps_out = ctx.enter_context(tc.tile_pool(name="ps_out", bufs=2, space="PSUM"))
        ps_tr = ctx.enter_context(tc.tile_pool(name="ps_tr", bufs=1, space="PSUM"))
        ps_rs = ctx.enter_context(tc.tile_pool(name="ps_rs", bufs=1, space="PSUM"))

        # ---- constants ----
        ident = const.tile([P, P], f32, tag="ident")
        make_identity(nc, ident)
        cpack = const.tile([P, 16], f32, tag="cpack")
        nc.vector.memset(cpack[:, 0:1], 1.0)      # ones column (reduce rhs)
        nc.vector.memset(cpack[:, 1:2], LN_EPS)
        ones = cpack[:, 0:1]
        eps = cpack[:, 1:2]
        bias_col = {}
        bv_bc = None
        if apply_qkv_bias:
            for i, n in enumerate(("bq", "bk")):
                dst = cpack[:, 2 + 4 * i: 2 + 4 * (i + 1)]
                nc.sync.dma_start(dst, bias_d[n].rearrange("(o p) -> p o", p=P))
                bias_col[n] = dst
            bv_bc = const.tile([P, L], f32, tag="bv_bc")
            nc.sync.dma_start(bv_bc[:], bcast_p(bias_d["bv"]))
        if apply_gamma_beta:
            gb_pack = const.tile([P, 2, L], f32, tag="gb")
            nc.sync.dma_start(gb_pack[:, 0, :], bcast_p(gamma_d))
            nc.sync.dma_start(gb_pack[:, 1, :], bcast_p(beta_d))

        # ---- weights: load W[m, l] fp32, transpose -> WT[l_p, li, m] bf16 ----
        WT = {}
        for n in ("Wq", "Wk", "Wv"):
            wld = inp.tile([P, NLC, F], f32, tag="bh")
            nc.sync.dma_start(wld[:], w_d[n].rearrange("(o p) l -> p o l", p=P))
            wt = const.tile([P, NLC, L], bf16, tag=f"WT_{n}")
            for mi in range(NLC):
                pst = ps_tr.tile([P, F], f32, tag="tr")
                for li in range(NLC):
                    nc.tensor.transpose(pst[:, ts(li, P)],
                                        wld[:, mi, ts(li, P)], ident[:])
                nc.vector.tensor_copy(
                    wt[:, :, ts(mi, P)],
                    pst.rearrange("p (li f) -> p li f", f=P))
            WT[n] = wt

        # ---- per batch element ----
        for bi in [i % NB for i in range(NB * repeat)]:
            # a: load fp32 per l-chunk, cast to bf16 (GPSIMD)
            a_sb = inp.tile([P, NLC, C], f32, tag="a")
            a_bf = acts.tile([P, NLC, C], bf16, tag="a_bf")
            for li in range(NLC):
                nc.sync.dma_start(a_sb[:, li, :],
                                  a_d[bi, ds(li * P, P), :])
                nc.gpsimd.tensor_copy(a_bf[:, li, :], a_sb[:, li, :])
            apT = acts.tile([P, NDB, L], f16, tag="apT")

            def emit_apt(gb, a_sb=a_sb, apT=apT):
                pst = ps_tr.tile([P, F], f32, tag="tr", name="pst")
                for li in range(NLC):
                    nc.tensor.transpose(pst[:, ts(li, P)],
                                        a_sb[:, li, ts(gb, P)], ident[:])
                nc.vector.tensor_copy(apT[:, gb, :], pst[:])

            # a few up-front (they only need a_sb); the rest interleave into
            # the QKV loop as PE filler between PSUM-limited matmul groups
            apt_queue = list(range(NDB))
            for _ in range(4):
                emit_apt(apt_queue.pop(0))

            # b: load fp32 in halves, cast to bf16
            b_bf = acts.tile([P, NLC, C], bf16, tag="bpt", bufs=2)
            for h in range(2):
                b_sb = inp.tile([P, 2, C], f32, tag="bh")
                nc.sync.dma_start(
                    b_sb[:],
                    b_d[bi, ds(h * 2 * P, 2 * P), :].rearrange(
                        "(o p) c -> p o c", p=P))
                for li in range(2):
                    nc.gpsimd.tensor_copy(b_bf[:, h * 2 + li, :], b_sb[:, li, :])

            # qT[m_p, mi, c], kT[m_p, mi, c] (bf16)
            qT = acts.tile([P, NLC, C], bf16, tag="qT")
            kT = acts.tile([P, NLC, C], bf16, tag="kT")
            for wname, bname, src, dst in (("Wq", "bq", a_bf, qT),
                                           ("Wk", "bk", b_bf, kT)):
                for mi in range(NLC):
                    for ci in range(NCCH):
                        ps = ps_mm.tile([P, F], f32, tag="mm")
                        for li in range(NLC):
                            nc.tensor.matmul(ps[:],
                                             lhsT=WT[wname][:, li, ts(mi, P)],
                                             rhs=src[:, li, ts(ci, F)],
                                             start=(li == 0), stop=(li == NLC - 1))
                        dslice = dst[:, mi, ts(ci, F)]
                        if apply_qkv_bias:
                            nc.scalar.activation(dslice, ps[:], AF.Identity,
                                                 bias=bias_col[bname][:, mi:mi + 1])
                        elif (mi + ci) % 2 == 0:
                            nc.scalar.copy(dslice, ps[:])
                        else:
                            nc.vector.tensor_copy(dslice, ps[:])
                        if apt_queue:
                            emit_apt(apt_queue.pop(0))

            # v[d_p, di, m] (bf16)
            v_sb = acts.tile([P, NDB, L], bf16, tag="v")
            for di in range(NDB):
                ps = ps_mm.tile([P, F], f32, tag="mm")
                for li in range(NLC):
                    nc.tensor.matmul(ps[:], lhsT=b_bf[:, li, ts(di, P)],
                                     rhs=WT["Wv"][:, li, :],
                                     start=(li == 0), stop=(li == NLC - 1))
                if di % 2 == 0:
                    nc.scalar.copy(v_sb[:, di, :], ps[:])
                else:
                    nc.vector.tensor_copy(v_sb[:, di, :], ps[:])
                if apply_qkv_bias:
                    nc.vector.tensor_add(v_sb[:, di, :], v_sb[:, di, :],
                                         bv_bc[:, :])

            # ---- attention, c-chunk at a time ----
            for ci in range(NCCH):
                PT = acts.tile([P, NDB, F], bf16, tag="bpt", bufs=2)
                sumacc = small.tile([P, F], f32, tag="sumacc")
                for di in range(NDB):
                    ps = ps_mm.tile([P, F], f32, tag="mm")
                    for mi in range(NLC):
                        nc.tensor.matmul(ps[:], lhsT=kT[:, mi, ts(di, P)],
                                         rhs=qT[:, mi, ts(ci, F)],
                                         start=(mi == 0), stop=(mi == NLC - 1))
                    nc.scalar.activation(PT[:, di, :], ps[:], AF.Exp,
                                         scale=INV_SQRT_L)
                    if di == 0:
                        nc.vector.tensor_copy(sumacc[:], PT[:, di, :])
                    else:
                        nc.vector.tensor_add(sumacc[:], sumacc[:], PT[:, di, :])

                for cb in range(NCCH):
                    gb = ci * NCCH + cb
                    # rowsum over partitions for these 128 queries
                    psr = ps_rs.tile([P, 1], f32, tag="rs")
                    nc.tensor.matmul(psr[:], lhsT=sumacc[:, ts(cb, P)],
                                     rhs=ones, start=True, stop=True)
                    stats = small.tile([P, 16], f32, tag="stats")
                    rs = stats[:, 0:1]
                    nc.vector.reciprocal(rs, psr[:])
                    # PV
                    po = ps_out.tile([P, L], f32, tag="out")
                    for di in range(NDB):
                        nc.tensor.matmul(po[:], lhsT=PT[:, di, ts(cb, P)],
                                         rhs=v_sb[:, di, :],
                                         start=(di == 0), stop=(di == NDB - 1))
                    out_sb = outp.tile([P, L], f32, tag="out")
                    nc.vector.tensor_scalar_mul(out_sb[:], po[:], rs)
                    nc.vector.tensor_add(out_sb[:], out_sb[:], apT[:, gb, :])
                    # LayerNorm over free dim (L)
                    st6 = stats[:, 2:8]
                    mv = stats[:, 8:10]
                    rstd = stats[:, 10:11]
                    nc.vector.bn_stats(st6, out_sb[:])
                    nc.vector.bn_aggr(mv, st6)
                    nc.scalar.activation(rstd, mv[:, 1:2], AF.Sqrt, bias=eps)
                    nc.vector.reciprocal(rstd, rstd)
                    nc.vector.tensor_scalar(out_sb[:], out_sb[:],
                                            mv[:, 0:1], rstd,
                                            ALU.subtract, ALU.mult)
                    if apply_gamma_beta:
                        nc.vector.tensor_mul(out_sb[:], out_sb[:],
                                             gb_pack[:, 0, :])
                        nc.vector.tensor_add(out_sb[:], out_sb[:],
                                             gb_pack[:, 1, :])
                    nc.sync.dma_start(out_d[bi, ds(gb * P, P), :], out_sb[:])

    nc.compile()
    return nc


def _get_nc(apply_qkv_bias, apply_gamma_beta, repeat=1):
    key = (apply_qkv_bias, apply_gamma_beta, repeat)
    if key not in _CACHE:
        _CACHE[key] = _build(*key)
    return _CACHE[key]


def _run(inputs, trace=False):
    from concourse import bass_utils

    a = np.ascontiguousarray(np.asarray(inputs["a"], dtype=np.float32))
    b = np.ascontiguousarray(np.asarray(inputs["b"], dtype=np.float32))
    get = lambda n: np.ascontiguousarray(np.asarray(inputs[n], dtype=np.float32))
    Wq, Wk, Wv = get("Wq"), get("Wk"), get("Wv")
    bq, bk, bv = get("bq"), get("bk"), get("bv")
    gamma, beta = get("gamma"), get("beta")

    apply_qkv_bias = bool(np.any(bq) or np.any(bk) or np.any(bv))
    apply_gamma_beta = bool(np.any(gamma != 1.0) or np.any(beta))
    nc = _get_nc(apply_qkv_bias, apply_gamma_beta)

    in_maps = []
    for c in range(NCORE):
        sl = slice(c * NB, (c + 1) * NB)
        in_maps.append({
            "a": np.ascontiguousarray(a[sl]), "b": np.ascontiguousarray(b[sl]),
            "Wq": Wq, "Wk": Wk, "Wv": Wv,
            "bq": bq, "bk": bk, "bv": bv,
            "gamma": gamma, "beta": beta,
        })
    res = bass_utils.run_bass_kernel_spmd(nc, in_maps,
                                          core_ids=list(range(NCORE)),
                                          trace=trace)
    out = np.concatenate(
        [res.results[c]["out"].reshape(NB, L, C) for c in range(NCORE)], axis=0)
    return out, res


def kernel(**inputs):
    out, _ = _run(inputs, trace=False)
    return out


# revision 18
# speedup vs baseline: 1.0957x; 1.0170x over previous
"""CrossDomainAttention TRN2 kernel: 8-core data-parallel over batch.

Reference computation (per batch element, a/b are (L, C) slices):
  ap = a.T (C, L);  q = ap@Wq.T+bq; k,v from b.T
  attn = softmax(q @ k.T / sqrt(L)) (C, C)
  out = LN(attn @ v + ap) over L, returned as the raw (C*L) buffer viewed (L, C)

Fast path (no qkv bias, no gamma/beta — the shipped inputs):
  scores = ap (Wq^T Wk) bp^T, so the q-projection disappears:
    GT = Wk^T Wq (precomputed once, fp8, x64)
    uT[l, d] = (GT^T b_raw)[l, d]  (fp8, keeps the x64)
    scoresT[d, c] = sum_l uT[l, d] a_raw[l, c]   (fp8 DoubleRow matmuls)
  PT[d, c] = exp(scoresT / (sqrt(L)*64)) stored fp8
  v8[d, m] = 16 * (b W_v^T)  fp8;  po = PT^T @ v8 (PSUM, fp8 DoubleRow)
  LayerNorm is scale-invariant per row, so skip the softmax division:
    x = po + (16*rowsum) * ap ;  out = (x - mean(x)) * rsqrt(var(x))
  rowsum via PE matmuls against a column of 16.0s.
"""

import numpy as np

B, L, C = 16, 512, 2048
NCORE = 8
NB = B // NCORE          # batch elements per core
P = 128
F = 512                  # matmul free-dim tile
NLC = L // P             # 4  l/m chunks
NDB = C // P             # 16 d-blocks / c-blocks
NCCH = C // F            # 4  c chunks
LN_EPS = 1e-5
SG = 64.0                # fp8 scale on GT
SV = 16.0                # fp8 scale on v (and the rowsum ones column)
ESC = 1.0 / (float(np.sqrt(L)) * SG)
INV_SQRT_L = 1.0 / float(np.sqrt(L))

_CACHE = {}


def _build_fast(repeat: int = 1):
    import concourse.bass as bass
    import concourse.tile as tile
    from concourse import bacc, mybir
    from concourse.bass import ts, ds
    from concourse.masks import make_identity
    from contextlib import ExitStack

    f32 = mybir.dt.float32
    f32r = mybir.dt.float32r
    f16 = mybir.dt.float16
    bf16 = mybir.dt.bfloat16
    f8 = mybir.dt.float8e4
    AF = mybir.ActivationFunctionType
    ALU = mybir.AluOpType
    DR = mybir.MatmulPerfMode.DoubleRow

    nc = bacc.Bacc("TRN2", target_bir_lowering=False, debug=False,
                   enable_asserts=False)

    a_d = nc.dram_tensor("a", (NB, L, C), f32, kind="ExternalInput").ap()
    b_d = nc.dram_tensor("b", (NB, L, C), f32, kind="ExternalInput").ap()
    w_d = {n: nc.dram_tensor(n, (L, L), f32, kind="ExternalInput").ap()
           for n in ("Wq", "Wk", "Wv")}
    for n in ("bq", "bk", "bv"):
        nc.dram_tensor(n, (L,), f32, kind="ExternalInput")
    nc.dram_tensor("gamma", (L,), f32, kind="ExternalInput")
    nc.dram_tensor("beta", (L,), f32, kind="ExternalInput")
    out_d = nc.dram_tensor("out", (NB, C, L), f32, kind="ExternalOutput").ap()

    with tile.TileContext(nc) as tc, ExitStack() as ctx:
        const = ctx.enter_context(tc.tile_pool(name="const", bufs=1))
        inp = ctx.enter_context(tc.tile_pool(name="inp", bufs=1))
        acts = ctx.enter_context(tc.tile_pool(name="acts", bufs=1))
        small = ctx.enter_context(tc.tile_pool(name="small", bufs=3))
        outp = ctx.enter_context(tc.tile_pool(name="outp", bufs=3))
        ps_mm = ctx.enter_context(tc.tile_pool(name="ps_mm", bufs=2, space="PSUM"))
        ps_out = ctx.enter_context(tc.tile_pool(name="ps_out", bufs=2, space="PSUM"))
        ps_tr = ctx.enter_context(tc.tile_pool(name="ps_tr", bufs=1, space="PSUM"))
        ps_rs = ctx.enter_context(tc.tile_pool(name="ps_rs", bufs=1, space="PSUM"))

        # ---- constants ----
        ident = const.tile([P, P], f32, tag="ident")
        make_identity(nc, ident)
        onesf = const.tile([P, 2], f32, tag="onesf")
        nc.vector.memset(onesf[:], SV)
        ones8 = const.tile([P, 2, 1], f8, tag="ones8")
        nc.vector.tensor_copy(ones8[:, :, 0], onesf[:])
        epsc = const.tile([P, 1], f32, tag="epsc")
        nc.vector.memset(epsc[:], LN_EPS)
        identb = const.tile([P, P], bf16, tag="identb")
        nc.vector.tensor_copy(identb[:], ident[:])

        # elem-0 input loads first: they gate the PE's first work
        pre_a = inp.tile([P, NLC, C], f32, tag="a_sb")
        pre_b = inp.tile([P, NLC, C], f32, tag="b_sb")
        for li in range(NLC):
            nc.sync.dma_start(pre_a[:, li, :], a_d[0, ds(li * P, P), :])
        wk_ld = inp.tile([P, NLC, L], f32, tag="wld")
        nc.sync.dma_start(wk_ld[:], w_d["Wk"].rearrange("(o p) l -> p o l", p=P))
        wq_ld = inp.tile([P, NLC, L], f32, tag="wld2")
        nc.sync.dma_start(wq_ld[:], w_d["Wq"].rearrange("(o p) l -> p o l", p=P))
        for li in range(NLC):
            nc.sync.dma_start(pre_b[:, li, :], b_d[0, ds(li * P, P), :])

        # elem-0 residual transposes are the PE's first possible work —
        # emit them before the weight matmuls (whose DMAs land later)
        apT0 = acts.tile([P, NDB, L], f16, tag="apT")
        for gb in range(NDB):
            pst = ps_tr.tile([P, F], f32, tag="tr")
            for li in range(NLC):
                nc.tensor.transpose(pst[:, ts(li, P)],
                                    pre_a[:, li, ts(gb, P)], ident[:])
            nc.vector.tensor_copy(apT0[:, gb, :], pst[:])

        # ---- weights ----
        # GT8 = SG * (Wk^T Wq): contraction over m using raw [m, l] layouts.
        wk_bf = inp.tile([P, NLC, L], bf16, tag="wbf")
        nc.vector.tensor_copy(wk_bf[:], wk_ld[:])
        wq_bf = inp.tile([P, NLC, L], bf16, tag="wbf2")
        nc.vector.tensor_copy(wq_bf[:], wq_ld[:])
        GT8 = const.tile([P, NLC, L], f8, tag="GT8")
        for lb in range(NLC):
            ps = ps_out.tile([P, F], f32, tag="out")
            for mi in range(NLC):
                nc.tensor.matmul(ps[:], lhsT=wk_bf[:, mi, ts(lb, P)],
                                 rhs=wq_bf[:, mi, :],
                                 start=(mi == 0), stop=(mi == NLC - 1))
            nc.scalar.mul(GT8[:, lb, :], ps[:], SG)
        # WvT8 = SV * Wv^T (PE transpose of raw Wv)
        wv_ld = inp.tile([P, NLC, L], f32, tag="wld")
        nc.sync.dma_start(wv_ld[:], w_d["Wv"].rearrange("(o p) l -> p o l", p=P))
        WvT8 = const.tile([P, NLC, L], f8, tag="WvT8")
        for li in range(NLC):
            pst = ps_tr.tile([P, F], f32, tag="tr")
            for mi in range(NLC):
                nc.tensor.transpose(pst[:, ts(mi, P)],
                                    wv_ld[:, mi, ts(li, P)], ident[:])
            nc.scalar.mul(WvT8[:, li, :], pst[:], SV)

        # ---- per batch element ----
        for idx, bi in enumerate([i % NB for i in range(NB * repeat)]):
            # loads + fp8 casts, spread across engines so the PE isn't
            # gated on one slow GpSimd cast chain at startup
            if idx == 0:
                a_sb, b_sb = pre_a, pre_b
                cast_eng = (nc.vector, nc.scalar, nc.vector, nc.scalar)
            else:
                a_sb = inp.tile([P, NLC, C], f32, tag="a_sb")
                b_sb = inp.tile([P, NLC, C], f32, tag="b_sb")
                cast_eng = (nc.vector, nc.scalar, nc.gpsimd, nc.gpsimd)
            a8 = acts.tile([P, NLC, C], f8, tag="a8", bufs=2)
            b8 = acts.tile([P, NLC, C], f8, tag="b8", bufs=2)

            def cast(eng, dst, src):
                if eng is nc.scalar:
                    eng.copy(dst, src)
                else:
                    eng.tensor_copy(dst, src)

            for li in range(NLC):
                if idx > 0:
                    nc.sync.dma_start(a_sb[:, li, :], a_d[bi, ds(li * P, P), :])
                cast(cast_eng[li], a8[:, li, :], a_sb[:, li, :])
            for li in range(NLC):
                if idx > 0:
                    nc.sync.dma_start(b_sb[:, li, :], b_d[bi, ds(li * P, P), :])
                cast(cast_eng[li], b8[:, li, :], b_sb[:, li, :])

            # apT[c_p, gb, m] = a[m, c] transposed (residual, f16)
            if idx == 0:
                apT = apT0
            else:
                apT = acts.tile([P, NDB, L], f16, tag="apT")
                for gb in range(NDB):
                    pst = ps_tr.tile([P, F], f32, tag="tr")
                    for li in range(NLC):
                        nc.tensor.transpose(pst[:, ts(li, P)],
                                            a_sb[:, li, ts(gb, P)], ident[:])
                    nc.vector.tensor_copy(apT[:, gb, :], pst[:])

            # uT[l_p, lb, d] = (GT^T b)[l, d], fp8 (keeps x SG)
            uT8 = acts.tile([P, NLC, C], f8, tag="uT8")
            for lb in range(NLC):
                for dc2 in range(0, NCCH, 2):
                    ps = ps_mm.tile([P, 2, F], f32, tag="mm")
                    for h in range(2):
                        for kp in range(0, NLC, 2):
                            nc.tensor.matmul(
                                ps[:, h, :],
                                lhsT=GT8[:, kp:kp + 2, ts(lb, P)],
                                rhs=b8[:, kp:kp + 2, ts(dc2 + h, F)],
                                start=(kp == 0), stop=(kp == NLC - 2),
                                perf_mode=DR)
                    nc.scalar.copy(uT8[:, lb, ds(dc2 * F, 2 * F)],
                                   ps.rearrange("p two f -> p (two f)"))

            # v8[d_p, di, m] = SV * (b Wv^T), fp8
            v8 = acts.tile([P, NDB, L], f8, tag="v8")
            for di2 in range(0, NDB, 2):
                ps = ps_mm.tile([P, 2, F], f32, tag="mm")
                for h in range(2):
                    for kp in range(0, NLC, 2):
                        nc.tensor.matmul(
                            ps[:, h, :],
                            lhsT=b8[:, kp:kp + 2, ts(di2 + h, P)],
                            rhs=WvT8[:, kp:kp + 2, :],
                            start=(kp == 0), stop=(kp == NLC - 2),
                            perf_mode=DR)
                nc.scalar.copy(
                    v8.rearrange("p n m -> p (n m)")[:, ds(di2 * F, 2 * F)],
                    ps.rearrange("p two f -> p (two f)"))

            # ---- attention, c-chunk at a time (PV/LN pipelined one behind) ----
            def emit_scores(ci):
                PT8 = acts.tile([P, NDB, F], f8, tag="PT", bufs=2)
                psr4 = small.tile([P, NCCH], f32, tag="psr", bufs=2)
                for di2 in range(0, NDB, 2):
                    ps = ps_mm.tile([P, 2, F], f32, tag="mm")
                    for h in range(2):
                        for kp in range(0, NLC, 2):
                            nc.tensor.matmul(
                                ps[:, h, :],
                                lhsT=uT8[:, kp:kp + 2, ts(di2 + h, P)],
                                rhs=a8[:, kp:kp + 2, ts(ci, F)],
                                start=(kp == 0), stop=(kp == NLC - 2),
                                perf_mode=DR)
                    nc.scalar.activation(
                        PT8.rearrange("p n f -> p (n f)")[:, ds(di2 * F, 2 * F)],
                        ps.rearrange("p two f -> p (two f)"), AF.Exp, scale=ESC)
                # psr[c] = SV * rowsum, per 128-query block
                for cb in range(NCCH):
                    psp = ps_rs.tile([P, 1], f32, tag="rs")
                    for di2 in range(0, NDB, 2):
                        nc.tensor.matmul(psp[:],
                                         lhsT=PT8[:, di2:di2 + 2, ts(cb, P)],
                                         rhs=ones8[:],
                                         start=(di2 == 0), stop=(di2 == NDB - 2),
                                         perf_mode=DR)
                    nc.vector.tensor_copy(psr4[:, cb:cb + 1], psp[:])
                return PT8, psr4

            def emit_pv_ln(ci, PT8, psr4, tail=False):
                mv4 = small.tile([P, NCCH, 2], f32, tag="mv4", bufs=2)
                xs = []
                for cb in range(NCCH):
                    po = ps_out.tile([P, L], f32, tag="out")
                    for di2 in range(0, NDB, 2):
                        nc.tensor.matmul(po[:],
                                         lhsT=PT8[:, di2:di2 + 2, ts(cb, P)],
                                         rhs=v8[:, di2:di2 + 2, :],
                                         start=(di2 == 0), stop=(di2 == NDB - 2),
                                         perf_mode=DR)
                    x_sb = outp.tile([P, L], f32, tag="x", bufs=6)
                    nc.vector.affine_then_add(x_sb[:], in0=apT[:, ci * NCCH + cb, :],
                                              in1=po[:], scale=psr4[:, cb:cb + 1],
                                              bias=0.0)
                    st6 = small.tile([P, 6], f32, tag="st6")
                    nc.vector.bn_stats(st6[:], x_sb[:])
                    nc.vector.bn_aggr(mv4[:, cb, :], st6[:])
                    xs.append(x_sb)
                    if tail:
                        # per-cb finish: shortest serial chain at kernel end
                        sdc = small.tile([P, 2], f32, tag="sdc")
                        nc.scalar.activation(sdc[:, 0:1], mv4[:, cb, 1:2],
                                             AF.Sqrt, bias=epsc[:])
                        nc.vector.reciprocal(sdc[:, 1:2], sdc[:, 0:1])
                        out_sb = outp.tile([P, L], f32, tag="osb", bufs=3)
                        nc.vector.tensor_scalar(out_sb[:], x_sb[:],
                                                mv4[:, cb, 0:1], sdc[:, 1:2],
                                                ALU.subtract, ALU.mult)
                        nc.sync.dma_start(out_d[bi, ds((ci * NCCH + cb) * P, P), :],
                                          out_sb[:])
                if tail:
                    return
                sd4 = small.tile([P, 2 * NCCH], f32, tag="sd4")
                nc.scalar.activation(sd4[:, 0:NCCH], mv4[:, :, 1], AF.Sqrt,
                                     bias=epsc[:])
                nc.vector.reciprocal(sd4[:, NCCH:2 * NCCH], sd4[:, 0:NCCH])
                for cb in range(NCCH):
                    gb = ci * NCCH + cb
                    out_sb = outp.tile([P, L], f32, tag="osb", bufs=3)
                    nc.vector.tensor_scalar(out_sb[:], xs[cb][:],
                                            mv4[:, cb, 0:1],
                                            sd4[:, NCCH + cb:NCCH + cb + 1],
                                            ALU.subtract, ALU.mult)
                    nc.sync.dma_start(out_d[bi, ds(gb * P, P), :], out_sb[:])

            prev = None
            for ci in range(NCCH):
                cur = emit_scores(ci)
                if prev is not None:
                    emit_pv_ln(ci - 1, *prev)
                prev = cur
            emit_pv_ln(NCCH - 1, *prev, tail=(idx == NB * repeat - 1))

    nc.compile()
    return nc


def _build(apply_qkv_bias: bool, apply_gamma_beta: bool, repeat: int = 1):
    if not apply_qkv_bias and not apply_gamma_beta:
        return _build_fast(repeat)
    return _build_generic(apply_qkv_bias, apply_gamma_beta, repeat)


def _build_generic(apply_qkv_bias: bool, apply_gamma_beta: bool, repeat: int = 1):
    import concourse.bass as bass
    import concourse.tile as tile
    from concourse import bacc, mybir
    from concourse.bass import ts, ds
    from concourse.masks import make_identity
    from contextlib import ExitStack

    f32 = mybir.dt.float32
    f16 = mybir.dt.float16
    bf16 = mybir.dt.bfloat16
    AF = mybir.ActivationFunctionType
    ALU = mybir.AluOpType

    nc = bacc.Bacc("TRN2", target_bir_lowering=False, debug=False,
                   enable_asserts=False)

    a_d = nc.dram_tensor("a", (NB, L, C), f32, kind="ExternalInput").ap()
    b_d = nc.dram_tensor("b", (NB, L, C), f32, kind="ExternalInput").ap()
    w_d = {n: nc.dram_tensor(n, (L, L), f32, kind="ExternalInput").ap()
           for n in ("Wq", "Wk", "Wv")}
    bias_d = {n: nc.dram_tensor(n, (L,), f32, kind="ExternalInput").ap()
              for n in ("bq", "bk", "bv")}
    gamma_d = nc.dram_tensor("gamma", (L,), f32, kind="ExternalInput").ap()
    beta_d = nc.dram_tensor("beta", (L,), f32, kind="ExternalInput").ap()
    out_d = nc.dram_tensor("out", (NB, C, L), f32, kind="ExternalOutput").ap()

    def bcast_p(ap1d):
        # broadcast a 1-D DRAM AP across all 128 partitions (DMA source)
        return bass.AP(tensor=ap1d.tensor, offset=ap1d.offset,
                       ap=[[0, P]] + [list(d) for d in ap1d.ap])

    with tile.TileContext(nc) as tc, ExitStack() as ctx:
        const = ctx.enter_context(tc.tile_pool(name="const", bufs=1))
        inp = ctx.enter_context(tc.tile_pool(name="inp", bufs=1))
        acts = ctx.enter_context(tc.tile_pool(name="acts", bufs=1))
        small = ctx.enter_context(tc.tile_pool(name="small", bufs=3))
        outp = ctx.enter_context(tc.tile_pool(name="outp", bufs=3))
        ps_mm = ctx.enter_context(tc.tile_pool(name="ps_mm", bufs=4, space="PSUM"))
        ps_out = ctx.enter_context(tc.tile_pool(name="ps_out", bufs=2, space="PSUM"))
        ps_tr = ctx.enter_context(tc.tile_pool(name="ps_tr", bufs=1, space="PSUM"))
        ps_rs = ctx.enter_context(tc.tile_pool(name="ps_rs", bufs=1, space="PSUM"))

        # ---- constants ----
        ident = const.tile([P, P], f32, tag="ident")
        make_identity(nc, ident)
        cpack = const.tile([P, 16], f32, tag="cpack")
        nc.vector.memset(cpack[:, 0:1], 1.0)      # ones column (reduce rhs)
        nc.vector.memset(cpack[:, 1:2], LN_EPS)
        ones = cpack[:, 0:1]
        eps = cpack[:, 1:2]
        bias_col = {}
        bv_bc = None
        if apply_qkv_bias:
            for i, n in enumerate(("bq", "bk")):
                dst = cpack[:, 2 + 4 * i: 2 + 4 * (i + 1)]
                nc.sync.dma_start(dst, bias_d[n].rearrange("(o p) -> p o", p=P))
                bias_col[n] = dst
            bv_bc = const.tile([P, L], f32, tag="bv_bc")
            nc.sync.dma_start(bv_bc[:], bcast_p(bias_d["bv"]))
        if apply_gamma_beta:
            gb_pack = const.tile([P, 2, L], f32, tag="gb")
            nc.sync.dma_start(gb_pack[:, 0, :], bcast_p(gamma_d))
            nc.sync.dma_start(gb_pack[:, 1, :], bcast_p(beta_d))

        # ---- weights: load W[m, l] fp32, transpose -> WT[l_p, li, m] bf16 ----
        WT = {}
        for n in ("Wq", "Wk", "Wv"):
            wld = inp.tile([P, NLC, F], f32, tag="bh")
            nc.sync.dma_start(wld[:], w_d[n].rearrange("(o p) l -> p o l", p=P))
            wt = const.tile([P, NLC, L], bf16, tag=f"WT_{n}")
            for mi in range(NLC):
                pst = ps_tr.tile([P, F], f32, tag="tr")
                for li in range(NLC):
                    nc.tensor.transpose(pst[:, ts(li, P)],
                                        wld[:, mi, ts(li, P)], ident[:])
                nc.vector.tensor_copy(
                    wt[:, :, ts(mi, P)],
                    pst.rearrange("p (li f) -> p li f", f=P))
            WT[n] = wt

        # ---- per batch element ----
        for bi in [i % NB for i in range(NB * repeat)]:
            # a: load fp32 per l-chunk, cast to bf16 (GPSIMD)
            a_sb = inp.tile([P, NLC, C], f32, tag="a")
            a_bf = acts.tile([P, NLC, C], bf16, tag="a_bf")
            for li in range(NLC):
                nc.sync.dma_start(a_sb[:, li, :],
                                  a_d[bi, ds(li * P, P), :])
                nc.gpsimd.tensor_copy(a_bf[:, li, :], a_sb[:, li, :])
            apT = acts.tile([P, NDB, L], f16, tag="apT")

            def emit_apt(gb, a_sb=a_sb, apT=apT):
                pst = ps_tr.tile([P, F], f32, tag="tr", name="pst")
                for li in range(NLC):
                    nc.tensor.transpose(pst[:, ts(li, P)],
                                        a_sb[:, li, ts(gb, P)], ident[:])
                nc.vector.tensor_copy(apT[:, gb, :], pst[:])

            # a few up-front (they only need a_sb); the rest interleave into
            # the QKV loop as PE filler between PSUM-limited matmul groups
            apt_queue = list(range(NDB))
            for _ in range(4):
                emit_apt(apt_queue.pop(0))

            # b: load fp32 in halves, cast to bf16
            b_bf = acts.tile([P, NLC, C], bf16, tag="bpt", bufs=2)
            for h in range(2):
                b_sb = inp.tile([P, 2, C], f32, tag="bh")
                nc.sync.dma_start(
                    b_sb[:],
                    b_d[bi, ds(h * 2 * P, 2 * P), :].rearrange(
                        "(o p) c -> p o c", p=P))
                for li in range(2):
                    nc.gpsimd.tensor_copy(b_bf[:, h * 2 + li, :], b_sb[:, li, :])

            # qT[m_p, mi, c], kT[m_p, mi, c] (bf16)
            qT = acts.tile([P, NLC, C], bf16, tag="qT")
            kT = acts.tile([P, NLC, C], bf16, tag="kT")
            for wname, bname, src, dst in (("Wq", "bq", a_bf, qT),
                                           ("Wk", "bk", b_bf, kT)):
                for mi in range(NLC):
                    for ci in range(NCCH):
                        ps = ps_mm.tile([P, F], f32, tag="mm")
                        for li in range(NLC):
                            nc.tensor.matmul(ps[:],
                                             lhsT=WT[wname][:, li, ts(mi, P)],
                                             rhs=src[:, li, ts(ci, F)],
                                             start=(li == 0), stop=(li == NLC - 1))
                        dslice = dst[:, mi, ts(ci, F)]
                        if apply_qkv_bias:
                            nc.scalar.activation(dslice, ps[:], AF.Identity,
                                                 bias=bias_col[bname][:, mi:mi + 1])
                        elif (mi + ci) % 2 == 0:
                            nc.scalar.copy(dslice, ps[:])
                        else:
                            nc.vector.tensor_copy(dslice, ps[:])
                        if apt_queue:
                            emit_apt(apt_queue.pop(0))

            # v[d_p, di, m] (bf16)
            v_sb = acts.tile([P, NDB, L], bf16, tag="v")
            for di in range(NDB):
                ps = ps_mm.tile([P, F], f32, tag="mm")
                for li in range(NLC):
                    nc.tensor.matmul(ps[:], lhsT=b_bf[:, li, ts(di, P)],
                                     rhs=WT["Wv"][:, li, :],
                                     start=(li == 0), stop=(li == NLC - 1))
                if di % 2 == 0:
                    nc.scalar.copy(v_sb[:, di, :], ps[:])
                else:
                    nc.vector.tensor_copy(v_sb[:, di, :], ps[:])
                if apply_qkv_bias:
                    nc.vector.tensor_add(v_sb[:, di, :], v_sb[:, di, :],
                                         bv_bc[:, :])

            # ---- attention, c-chunk at a time ----
            for ci in range(NCCH):
                PT = acts.tile([P, NDB, F], bf16, tag="bpt", bufs=2)
                sumacc = small.tile([P, F], f32, tag="sumacc")
                for di in range(NDB):
                    ps = ps_mm.tile([P, F], f32, tag="mm")
                    for mi in range(NLC):
                        nc.tensor.matmul(ps[:], lhsT=kT[:, mi, ts(di, P)],
                                         rhs=qT[:, mi, ts(ci, F)],
                                         start=(mi == 0), stop=(mi == NLC - 1))
                    nc.scalar.activation(PT[:, di, :], ps[:], AF.Exp,
                                         scale=INV_SQRT_L)
                    if di == 0:
                        nc.vector.tensor_copy(sumacc[:], PT[:, di, :])
                    else:
                        nc.vector.tensor_add(sumacc[:], sumacc[:], PT[:, di, :])

                for cb in range(NCCH):
                    gb = ci * NCCH + cb
                    # rowsum over partitions for these 128 queries
                    psr = ps_rs.tile([P, 1], f32, tag="rs")
                    nc.tensor.matmul(psr[:], lhsT=sumacc[:, ts(cb, P)],
                                     rhs=ones, start=True, stop=True)
                    stats = small.tile([P, 16], f32, tag="stats")
                    rs = stats[:, 0:1]
                    nc.vector.reciprocal(rs, psr[:])
                    # PV
                    po = ps_out.tile([P, L], f32, tag="out")
                    for di in range(NDB):
                        nc.tensor.matmul(po[:], lhsT=PT[:, di, ts(cb, P)],
                                         rhs=v_sb[:, di, :],
                                         start=(di == 0), stop=(di == NDB - 1))
                    out_sb = outp.tile([P, L], f32, tag="out")
                    nc.vector.tensor_scalar_mul(out_sb[:], po[:], rs)
                    nc.vector.tensor_add(out_sb[:], out_sb[:], apT[:, gb, :])
                    # LayerNorm over free dim (L)
                    st6 = stats[:, 2:8]
                    mv = stats[:, 8:10]
                    rstd = stats[:, 10:11]
                    nc.vector.bn_stats(st6, out_sb[:])
                    nc.vector.bn_aggr(mv, st6)
                    nc.scalar.activation(rstd, mv[:, 1:2], AF.Sqrt, bias=eps)
                    nc.vector.reciprocal(rstd, rstd)
                    nc.vector.tensor_scalar(out_sb[:], out_sb[:],
                                            mv[:, 0:1], rstd,
                                            ALU.subtract, ALU.mult)
                    if apply_gamma_beta:
                        nc.vector.tensor_mul(out_sb[:], out_sb[:],
                                             gb_pack[:, 0, :])
                        nc.vector.tensor_add(out_sb[:], out_sb[:],
                                             gb_pack[:, 1, :])
                    nc.sync.dma_start(out_d[bi, ds(gb * P, P), :], out_sb[:])

    nc.compile()
    return nc


def _get_nc(apply_qkv_bias, apply_gamma_beta, repeat=1):
    key = (apply_qkv_bias, apply_gamma_beta, repeat)
    if key not in _CACHE:
        _CACHE[key] = _build(*key)
    return _CACHE[key]


def _run(inputs, trace=False):
    from concourse import bass_utils

    a = np.ascontiguousarray(np.asarray(inputs["a"], dtype=np.float32))
    b = np.ascontiguousarray(np.asarray(inputs["b"], dtype=np.float32))
    get = lambda n: np.ascontiguousarray(np.asarray(inputs[n], dtype=np.float32))
    Wq, Wk, Wv = get("Wq"), get("Wk"), get("Wv")
    bq, bk, bv = get("bq"), get("bk"), get("bv")
    gamma, beta = get("gamma"), get("beta")

    apply_qkv_bias = bool(np.any(bq) or np.any(bk) or np.any(bv))
    apply_gamma_beta = bool(np.any(gamma != 1.0) or np.any(beta))
    nc = _get_nc(apply_qkv_bias, apply_gamma_beta)

    in_maps = []
    for c in range(NCORE):
        sl = slice(c * NB, (c + 1) * NB)
        in_maps.append({
            "a": np.ascontiguousarray(a[sl]), "b": np.ascontiguousarray(b[sl]),
            "Wq": Wq, "Wk": Wk, "Wv": Wv,
            "bq": bq, "bk": bk, "bv": bv,
            "gamma": gamma, "beta": beta,
        })
    res = bass_utils.run_bass_kernel_spmd(nc, in_maps,
                                          core_ids=list(range(NCORE)),
                                          trace=trace)
    out = np.concatenate(
        [res.results[c]["out"].reshape(NB, L, C) for c in range(NCORE)], axis=0)
    return out, res


def kernel(**inputs):
    out, _ = _run(inputs, trace=False)
    return out


# revision 22
# speedup vs baseline: 1.1087x; 1.0118x over previous
"""CrossDomainAttention TRN2 kernel: 8-core data-parallel over batch.

Reference computation (per batch element, a/b are (L, C) slices):
  ap = a.T (C, L);  q = ap@Wq.T+bq; k,v from b.T
  attn = softmax(q @ k.T / sqrt(L)) (C, C)
  out = LN(attn @ v + ap) over L, returned as the raw (C*L) buffer viewed (L, C)

Fast path (no qkv bias, no gamma/beta — the shipped inputs):
  scores = ap (Wq^T Wk) bp^T, so the q-projection disappears:
    GT = Wk^T Wq (precomputed once, fp8, x64)
    uT[l, d] = (GT^T b_raw)[l, d]  (fp8, keeps the x64)
    scoresT[d, c] = sum_l uT[l, d] a_raw[l, c]   (fp8 DoubleRow matmuls)
  PT[d, c] = exp(scoresT / (sqrt(L)*64)) stored fp8
  v8[d, m] = 16 * (b W_v^T)  fp8;  po = PT^T @ v8 (PSUM, fp8 DoubleRow)
  LayerNorm is scale-invariant per row, so skip the softmax division:
    x = po + (16*rowsum) * ap ;  out = (x - mean(x)) * rsqrt(var(x))
  rowsum via PE matmuls against a column of 16.0s.
"""

import numpy as np

B, L, C = 16, 512, 2048
NCORE = 8
NB = B // NCORE          # batch elements per core
P = 128
F = 512                  # matmul free-dim tile
NLC = L // P             # 4  l/m chunks
NDB = C // P             # 16 d-blocks / c-blocks
NCCH = C // F            # 4  c chunks
LN_EPS = 1e-5
SG = 64.0                # fp8 scale on GT
SV = 16.0                # fp8 scale on v (and the rowsum ones column)
ESC = 1.0 / (float(np.sqrt(L)) * SG)
INV_SQRT_L = 1.0 / float(np.sqrt(L))

_CACHE = {}


def _build_fast(repeat: int = 1):
    import concourse.bass as bass
    import concourse.tile as tile
    from concourse import bacc, mybir
    from concourse.bass import ts, ds
    from concourse.masks import make_identity
    from contextlib import ExitStack

    f32 = mybir.dt.float32
    f32r = mybir.dt.float32r
    f16 = mybir.dt.float16
    bf16 = mybir.dt.bfloat16
    f8 = mybir.dt.float8e4
    AF = mybir.ActivationFunctionType
    ALU = mybir.AluOpType
    DR = mybir.MatmulPerfMode.DoubleRow

    nc = bacc.Bacc("TRN2", target_bir_lowering=False, debug=False,
                   enable_asserts=False)

    a_d = nc.dram_tensor("a", (NB, L, C), f32, kind="ExternalInput").ap()
    b_d = nc.dram_tensor("b", (NB, L, C), f32, kind="ExternalInput").ap()
    w_d = {n: nc.dram_tensor(n, (L, L), f32, kind="ExternalInput").ap()
           for n in ("Wq", "Wk", "Wv")}
    for n in ("bq", "bk", "bv"):
        nc.dram_tensor(n, (L,), f32, kind="ExternalInput")
    nc.dram_tensor("gamma", (L,), f32, kind="ExternalInput")
    nc.dram_tensor("beta", (L,), f32, kind="ExternalInput")
    out_d = nc.dram_tensor("out", (NB, C, L), f32, kind="ExternalOutput").ap()

    with tile.TileContext(nc) as tc, ExitStack() as ctx:
        const = ctx.enter_context(tc.tile_pool(name="const", bufs=1))
        inp = ctx.enter_context(tc.tile_pool(name="inp", bufs=1))
        acts = ctx.enter_context(tc.tile_pool(name="acts", bufs=1))
        small = ctx.enter_context(tc.tile_pool(name="small", bufs=3))
        outp = ctx.enter_context(tc.tile_pool(name="outp", bufs=3))
        ps_mm = ctx.enter_context(tc.tile_pool(name="ps_mm", bufs=2, space="PSUM"))
        ps_out = ctx.enter_context(tc.tile_pool(name="ps_out", bufs=2, space="PSUM"))
        ps_tr = ctx.enter_context(tc.tile_pool(name="ps_tr", bufs=1, space="PSUM"))
        ps_rs = ctx.enter_context(tc.tile_pool(name="ps_rs", bufs=1, space="PSUM"))

        # ---- constants ----
        ident = const.tile([P, P], f32, tag="ident")
        make_identity(nc, ident)
        onesf = const.tile([P, 2], f32, tag="onesf")
        nc.vector.memset(onesf[:], SV)
        ones8 = const.tile([P, 2, 1], f8, tag="ones8")
        nc.vector.tensor_copy(ones8[:, :, 0], onesf[:])
        epsc = const.tile([P, 1], f32, tag="epsc")
        nc.vector.memset(epsc[:], LN_EPS)
        identb = const.tile([P, P], bf16, tag="identb")
        nc.vector.tensor_copy(identb[:], ident[:])

        # elem-0 input loads first: they gate the PE's first work
        pre_a = inp.tile([P, NLC, C], f32, tag="a_sb")
        pre_b = inp.tile([P, NLC, C], f32, tag="b_sb")
        for li in range(NLC):
            nc.sync.dma_start(pre_a[:, li, :], a_d[0, ds(li * P, P), :])
        wk_ld = inp.tile([P, NLC, L], f32, tag="wld")
        nc.sync.dma_start(wk_ld[:], w_d["Wk"].rearrange("(o p) l -> p o l", p=P))
        wq_ld = inp.tile([P, NLC, L], f32, tag="wld2")
        nc.sync.dma_start(wq_ld[:], w_d["Wq"].rearrange("(o p) l -> p o l", p=P))
        for li in range(NLC):
            nc.sync.dma_start(pre_b[:, li, :], b_d[0, ds(li * P, P), :])

        # elem-0 residual transposes are the PE's first possible work —
        # emit them before the weight matmuls (whose DMAs land later)
        apT0 = acts.tile([P, NDB, L], f16, tag="apT")
        for gb in range(NDB):
            # alternate PSUM banks so the PE never waits on the DVE drain
            if gb % 2 == 0:
                pst = ps_tr.tile([P, F], f32, tag="tr", name="pst")
            else:
                pst = ps_out.tile([P, L], f32, tag="out", name="pst")
            for li in range(NLC):
                nc.tensor.transpose(pst[:, ts(li, P)],
                                    pre_a[:, li, ts(gb, P)], ident[:])
            nc.vector.tensor_copy(apT0[:, gb, :], pst[:])

        # ---- weights ----
        # GT8 = SG * (Wk^T Wq): contraction over m using raw [m, l] layouts.
        wk_bf = inp.tile([P, NLC, L], bf16, tag="wbf")
        nc.vector.tensor_copy(wk_bf[:], wk_ld[:])
        wq_bf = inp.tile([P, NLC, L], bf16, tag="wbf2")
        nc.vector.tensor_copy(wq_bf[:], wq_ld[:])
        GT8 = const.tile([P, NLC, L], f8, tag="GT8")
        for lb in range(NLC):
            ps = ps_out.tile([P, F], f32, tag="out")
            for mi in range(NLC):
                nc.tensor.matmul(ps[:], lhsT=wk_bf[:, mi, ts(lb, P)],
                                 rhs=wq_bf[:, mi, :],
                                 start=(mi == 0), stop=(mi == NLC - 1))
            nc.scalar.mul(GT8[:, lb, :], ps[:], SG)
        # WvT8 = SV * Wv^T (PE transpose of raw Wv)
        wv_ld = inp.tile([P, NLC, L], f32, tag="wld")
        nc.sync.dma_start(wv_ld[:], w_d["Wv"].rearrange("(o p) l -> p o l", p=P))
        WvT8 = const.tile([P, NLC, L], f8, tag="WvT8")
        for li in range(NLC):
            pst = ps_tr.tile([P, F], f32, tag="tr")
            for mi in range(NLC):
                nc.tensor.transpose(pst[:, ts(mi, P)],
                                    wv_ld[:, mi, ts(li, P)], ident[:])
            nc.scalar.mul(WvT8[:, li, :], pst[:], SV)

        # ---- per batch element ----
        for idx, bi in enumerate([i % NB for i in range(NB * repeat)]):
            # loads + fp8 casts, spread across engines so the PE isn't
            # gated on one slow GpSimd cast chain at startup
            if idx == 0:
                a_sb, b_sb = pre_a, pre_b
                cast_eng = (nc.vector, nc.scalar, nc.vector, nc.scalar)
            else:
                a_sb = inp.tile([P, NLC, C], f32, tag="a_sb")
                b_sb = inp.tile([P, NLC, C], f32, tag="b_sb")
                cast_eng = (nc.vector, nc.scalar, nc.gpsimd, nc.gpsimd)
            a8 = acts.tile([P, NLC, C], f8, tag="a8", bufs=2)
            b8 = acts.tile([P, NLC, C], f8, tag="b8", bufs=2)

            def cast(eng, dst, src):
                if eng is nc.scalar:
                    eng.copy(dst, src)
                else:
                    eng.tensor_copy(dst, src)

            for li in range(NLC):
                if idx > 0:
                    nc.sync.dma_start(a_sb[:, li, :], a_d[bi, ds(li * P, P), :])
                cast(cast_eng[li], a8[:, li, :], a_sb[:, li, :])
            for li in range(NLC):
                if idx > 0:
                    nc.sync.dma_start(b_sb[:, li, :], b_d[bi, ds(li * P, P), :])
                cast(cast_eng[li], b8[:, li, :], b_sb[:, li, :])

            # apT[c_p, gb, m] = a[m, c] transposed (residual, f16)
            if idx == 0:
                apT = apT0
            else:
                apT = acts.tile([P, NDB, L], f16, tag="apT")
                for gb in range(NDB):
                    if gb % 2 == 0:
                        pst = ps_tr.tile([P, F], f32, tag="tr", name="pst")
                    else:
                        pst = ps_out.tile([P, L], f32, tag="out", name="pst")
                    for li in range(NLC):
                        nc.tensor.transpose(pst[:, ts(li, P)],
                                            a_sb[:, li, ts(gb, P)], ident[:])
                    nc.vector.tensor_copy(apT[:, gb, :], pst[:])

            # uT[l_p, lb, d] = (GT^T b)[l, d], fp8 (keeps x SG)
            uT8 = acts.tile([P, NLC, C], f8, tag="uT8")
            for lb in range(NLC):
                for dc2 in range(0, NCCH, 2):
                    ps = ps_mm.tile([P, 2, F], f32, tag="mm")
                    for h in range(2):
                        for kp in range(0, NLC, 2):
                            nc.tensor.matmul(
                                ps[:, h, :],
                                lhsT=GT8[:, kp:kp + 2, ts(lb, P)],
                                rhs=b8[:, kp:kp + 2, ts(dc2 + h, F)],
                                start=(kp == 0), stop=(kp == NLC - 2),
                                perf_mode=DR)
                    nc.scalar.copy(uT8[:, lb, ds(dc2 * F, 2 * F)],
                                   ps.rearrange("p two f -> p (two f)"))

            # v8[d_p, di, m] = SV * (b Wv^T), fp8
            v8 = acts.tile([P, NDB, L], f8, tag="v8")
            for di2 in range(0, NDB, 2):
                ps = ps_mm.tile([P, 2, F], f32, tag="mm")
                for h in range(2):
                    for kp in range(0, NLC, 2):
                        nc.tensor.matmul(
                            ps[:, h, :],
                            lhsT=b8[:, kp:kp + 2, ts(di2 + h, P)],
                            rhs=WvT8[:, kp:kp + 2, :],
                            start=(kp == 0), stop=(kp == NLC - 2),
                            perf_mode=DR)
                nc.scalar.copy(
                    v8.rearrange("p n m -> p (n m)")[:, ds(di2 * F, 2 * F)],
                    ps.rearrange("p two f -> p (two f)"))

            # ---- attention, c-chunk at a time (PV/LN pipelined one behind) ----
            def emit_scores(ci):
                PT8 = acts.tile([P, NDB, F], f8, tag="PT", bufs=2)
                psr4 = small.tile([P, NCCH], f32, tag="psr", bufs=2)
                for di2 in range(0, NDB, 2):
                    ps = ps_mm.tile([P, 2, F], f32, tag="mm")
                    for h in range(2):
                        for kp in range(0, NLC, 2):
                            nc.tensor.matmul(
                                ps[:, h, :],
                                lhsT=uT8[:, kp:kp + 2, ts(di2 + h, P)],
                                rhs=a8[:, kp:kp + 2, ts(ci, F)],
                                start=(kp == 0), stop=(kp == NLC - 2),
                                perf_mode=DR)
                    nc.scalar.activation(
                        PT8.rearrange("p n f -> p (n f)")[:, ds(di2 * F, 2 * F)],
                        ps.rearrange("p two f -> p (two f)"), AF.Exp, scale=ESC)
                # psr[c] = SV * rowsum, per 128-query block
                for cb in range(NCCH):
                    psp = ps_rs.tile([P, 1], f32, tag="rs")
                    for di2 in range(0, NDB, 2):
                        nc.tensor.matmul(psp[:],
                                         lhsT=PT8[:, di2:di2 + 2, ts(cb, P)],
                                         rhs=ones8[:],
                                         start=(di2 == 0), stop=(di2 == NDB - 2),
                                         perf_mode=DR)
                    nc.vector.tensor_copy(psr4[:, cb:cb + 1], psp[:])
                return PT8, psr4

            def emit_pv_ln(ci, PT8, psr4, tail=False):
                mv4 = small.tile([P, NCCH, 2], f32, tag="mv4", bufs=2)
                xs = []
                for cb in range(NCCH):
                    po = ps_out.tile([P, L], f32, tag="out")
                    for di2 in range(0, NDB, 2):
                        nc.tensor.matmul(po[:],
                                         lhsT=PT8[:, di2:di2 + 2, ts(cb, P)],
                                         rhs=v8[:, di2:di2 + 2, :],
                                         start=(di2 == 0), stop=(di2 == NDB - 2),
                                         perf_mode=DR)
                    x_sb = outp.tile([P, L], f32, tag="x", bufs=6)
                    nc.vector.affine_then_add(x_sb[:], in0=apT[:, ci * NCCH + cb, :],
                                              in1=po[:], scale=psr4[:, cb:cb + 1],
                                              bias=0.0)
                    st6 = small.tile([P, 6], f32, tag="st6")
                    nc.vector.bn_stats(st6[:], x_sb[:])
                    nc.vector.bn_aggr(mv4[:, cb, :], st6[:])
                    xs.append(x_sb)
                    if tail:
                        # per-cb finish: shortest serial chain at kernel end
                        sdc = small.tile([P, 2], f32, tag="sdc")
                        nc.scalar.activation(sdc[:, 0:1], mv4[:, cb, 1:2],
                                             AF.Sqrt, bias=epsc[:])
                        nc.vector.reciprocal(sdc[:, 1:2], sdc[:, 0:1])
                        out_sb = outp.tile([P, L], f32, tag="osb", bufs=3)
                        nc.vector.tensor_scalar(out_sb[:], x_sb[:],
                                                mv4[:, cb, 0:1], sdc[:, 1:2],
                                                ALU.subtract, ALU.mult)
                        nc.sync.dma_start(out_d[bi, ds((ci * NCCH + cb) * P, P), :],
                                          out_sb[:])
                if tail:
                    return
                sd4 = small.tile([P, 2 * NCCH], f32, tag="sd4")
                nc.scalar.activation(sd4[:, 0:NCCH], mv4[:, :, 1], AF.Sqrt,
                                     bias=epsc[:])
                nc.vector.reciprocal(sd4[:, NCCH:2 * NCCH], sd4[:, 0:NCCH])
                for cb in range(NCCH):
                    gb = ci * NCCH + cb
                    out_sb = outp.tile([P, L], f32, tag="osb", bufs=3)
                    nc.vector.tensor_scalar(out_sb[:], xs[cb][:],
                                            mv4[:, cb, 0:1],
                                            sd4[:, NCCH + cb:NCCH + cb + 1],
                                            ALU.subtract, ALU.mult)
                    nc.sync.dma_start(out_d[bi, ds(gb * P, P), :], out_sb[:])

            prev = None
            for ci in range(NCCH):
                cur = emit_scores(ci)
                if prev is not None:
                    emit_pv_ln(ci - 1, *prev)
                prev = cur
            emit_pv_ln(NCCH - 1, *prev, tail=(idx == NB * repeat - 1))

    nc.compile()
    return nc


def _build(apply_qkv_bias: bool, apply_gamma_beta: bool, repeat: int = 1):
    if not apply_qkv_bias and not apply_gamma_beta:
        return _build_fast(repeat)
    return _build_generic(apply_qkv_bias, apply_gamma_beta, repeat)


def _build_generic(apply_qkv_bias: bool, apply_gamma_beta: bool, repeat: int = 1):
    import concourse.bass as bass
    import concourse.tile as tile
    from concourse import bacc, mybir
    from concourse.bass import ts, ds
    from concourse.masks import make_identity
    from contextlib import ExitStack

    f32 = mybir.dt.float32
    f16 = mybir.dt.float16
    bf16 = mybir.dt.bfloat16
    AF = mybir.ActivationFunctionType
    ALU = mybir.AluOpType

    nc = bacc.Bacc("TRN2", target_bir_lowering=False, debug=False,
                   enable_asserts=False)

    a_d = nc.dram_tensor("a", (NB, L, C), f32, kind="ExternalInput").ap()
    b_d = nc.dram_tensor("b", (NB, L, C), f32, kind="ExternalInput").ap()
    w_d = {n: nc.dram_tensor(n, (L, L), f32, kind="ExternalInput").ap()
           for n in ("Wq", "Wk", "Wv")}
    bias_d = {n: nc.dram_tensor(n, (L,), f32, kind="ExternalInput").ap()
              for n in ("bq", "bk", "bv")}
    gamma_d = nc.dram_tensor("gamma", (L,), f32, kind="ExternalInput").ap()
    beta_d = nc.dram_tensor("beta", (L,), f32, kind="ExternalInput").ap()
    out_d = nc.dram_tensor("out", (NB, C, L), f32, kind="ExternalOutput").ap()

    def bcast_p(ap1d):
        # broadcast a 1-D DRAM AP across all 128 partitions (DMA source)
        return bass.AP(tensor=ap1d.tensor, offset=ap1d.offset,
                       ap=[[0, P]] + [list(d) for d in ap1d.ap])

    with tile.TileContext(nc) as tc, ExitStack() as ctx:
        const = ctx.enter_context(tc.tile_pool(name="const", bufs=1))
        inp = ctx.enter_context(tc.tile_pool(name="inp", bufs=1))
        acts = ctx.enter_context(tc.tile_pool(name="acts", bufs=1))
        small = ctx.enter_context(tc.tile_pool(name="small", bufs=3))
        outp = ctx.enter_context(tc.tile_pool(name="outp", bufs=3))
        ps_mm = ctx.enter_context(tc.tile_pool(name="ps_mm", bufs=4, space="PSUM"))
        ps_out = ctx.enter_context(tc.tile_pool(name="ps_out", bufs=2, space="PSUM"))
        ps_tr = ctx.enter_context(tc.tile_pool(name="ps_tr", bufs=1, space="PSUM"))
        ps_rs = ctx.enter_context(tc.tile_pool(name="ps_rs", bufs=1, space="PSUM"))

        # ---- constants ----
        ident = const.tile([P, P], f32, tag="ident")
        make_identity(nc, ident)
        cpack = const.tile([P, 16], f32, tag="cpack")
        nc.vector.memset(cpack[:, 0:1], 1.0)      # ones column (reduce rhs)
        nc.vector.memset(cpack[:, 1:2], LN_EPS)
        ones = cpack[:, 0:1]
        eps = cpack[:, 1:2]
        bias_col = {}
        bv_bc = None
        if apply_qkv_bias:
            for i, n in enumerate(("bq", "bk")):
                dst = cpack[:, 2 + 4 * i: 2 + 4 * (i + 1)]
                nc.sync.dma_start(dst, bias_d[n].rearrange("(o p) -> p o", p=P))
                bias_col[n] = dst
            bv_bc = const.tile([P, L], f32, tag="bv_bc")
            nc.sync.dma_start(bv_bc[:], bcast_p(bias_d["bv"]))
        if apply_gamma_beta:
            gb_pack = const.tile([P, 2, L], f32, tag="gb")
            nc.sync.dma_start(gb_pack[:, 0, :], bcast_p(gamma_d))
            nc.sync.dma_start(gb_pack[:, 1, :], bcast_p(beta_d))

        # ---- weights: load W[m, l] fp32, transpose -> WT[l_p, li, m] bf16 ----
        WT = {}
        for n in ("Wq", "Wk", "Wv"):
            wld = inp.tile([P, NLC, F], f32, tag="bh")
            nc.sync.dma_start(wld[:], w_d[n].rearrange("(o p) l -> p o l", p=P))
            wt = const.tile([P, NLC, L], bf16, tag=f"WT_{n}")
            for mi in range(NLC):
                pst = ps_tr.tile([P, F], f32, tag="tr")
                for li in range(NLC):
                    nc.tensor.transpose(pst[:, ts(li, P)],
                                        wld[:, mi, ts(li, P)], ident[:])
                nc.vector.tensor_copy(
                    wt[:, :, ts(mi, P)],
                    pst.rearrange("p (li f) -> p li f", f=P))
            WT[n] = wt

        # ---- per batch element ----
        for bi in [i % NB for i in range(NB * repeat)]:
            # a: load fp32 per l-chunk, cast to bf16 (GPSIMD)
            a_sb = inp.tile([P, NLC, C], f32, tag="a")
            a_bf = acts.tile([P, NLC, C], bf16, tag="a_bf")
            for li in range(NLC):
                nc.sync.dma_start(a_sb[:, li, :],
                                  a_d[bi, ds(li * P, P), :])
                nc.gpsimd.tensor_copy(a_bf[:, li, :], a_sb[:, li, :])
            apT = acts.tile([P, NDB, L], f16, tag="apT")

            def emit_apt(gb, a_sb=a_sb, apT=apT):
                pst = ps_tr.tile([P, F], f32, tag="tr", name="pst")
                for li in range(NLC):
                    nc.tensor.transpose(pst[:, ts(li, P)],
                                        a_sb[:, li, ts(gb, P)], ident[:])
                nc.vector.tensor_copy(apT[:, gb, :], pst[:])

            # a few up-front (they only need a_sb); the rest interleave into
            # the QKV loop as PE filler between PSUM-limited matmul groups
            apt_queue = list(range(NDB))
            for _ in range(4):
                emit_apt(apt_queue.pop(0))

            # b: load fp32 in halves, cast to bf16
            b_bf = acts.tile([P, NLC, C], bf16, tag="bpt", bufs=2)
            for h in range(2):
                b_sb = inp.tile([P, 2, C], f32, tag="bh")
                nc.sync.dma_start(
                    b_sb[:],
                    b_d[bi, ds(h * 2 * P, 2 * P), :].rearrange(
                        "(o p) c -> p o c", p=P))
                for li in range(2):
                    nc.gpsimd.tensor_copy(b_bf[:, h * 2 + li, :], b_sb[:, li, :])

            # qT[m_p, mi, c], kT[m_p, mi, c] (bf16)
            qT = acts.tile([P, NLC, C], bf16, tag="qT")
            kT = acts.tile([P, NLC, C], bf16, tag="kT")
            for wname, bname, src, dst in (("Wq", "bq", a_bf, qT),
                                           ("Wk", "bk", b_bf, kT)):
                for mi in range(NLC):
                    for ci in range(NCCH):
                        ps = ps_mm.tile([P, F], f32, tag="mm")
                        for li in range(NLC):
                            nc.tensor.matmul(ps[:],
                                             lhsT=WT[wname][:, li, ts(mi, P)],
                                             rhs=src[:, li, ts(ci, F)],
                                             start=(li == 0), stop=(li == NLC - 1))
                        dslice = dst[:, mi, ts(ci, F)]
                        if apply_qkv_bias:
                            nc.scalar.activation(dslice, ps[:], AF.Identity,
                                                 bias=bias_col[bname][:, mi:mi + 1])
                        elif (mi + ci) % 2 == 0:
                            nc.scalar.copy(dslice, ps[:])
                        else:
                            nc.vector.tensor_copy(dslice, ps[:])
                        if apt_queue:
                            emit_apt(apt_queue.pop(0))

            # v[d_p, di, m] (bf16)
            v_sb = acts.tile([P, NDB, L], bf16, tag="v")
            for di in range(NDB):
                ps = ps_mm.tile([P, F], f32, tag="mm")
                for li in range(NLC):
                    nc.tensor.matmul(ps[:], lhsT=b_bf[:, li, ts(di, P)],
                                     rhs=WT["Wv"][:, li, :],
                                     start=(li == 0), stop=(li == NLC - 1))
                if di % 2 == 0:
                    nc.scalar.copy(v_sb[:, di, :], ps[:])
                else:
                    nc.vector.tensor_copy(v_sb[:, di, :], ps[:])
                if apply_qkv_bias:
                    nc.vector.tensor_add(v_sb[:, di, :], v_sb[:, di, :],
                                         bv_bc[:, :])

            # ---- attention, c-chunk at a time ----
            for ci in range(NCCH):
                PT = acts.tile([P, NDB, F], bf16, tag="bpt", bufs=2)
                sumacc = small.tile([P, F], f32, tag="sumacc")
                for di in range(NDB):
                    ps = ps_mm.tile([P, F], f32, tag="mm")
                    for mi in range(NLC):
                        nc.tensor.matmul(ps[:], lhsT=kT[:, mi, ts(di, P)],
                                         rhs=qT[:, mi, ts(ci, F)],
                                         start=(mi == 0), stop=(mi == NLC - 1))
                    nc.scalar.activation(PT[:, di, :], ps[:], AF.Exp,
                                         scale=INV_SQRT_L)
                    if di == 0:
                        nc.vector.tensor_copy(sumacc[:], PT[:, di, :])
                    else:
                        nc.vector.tensor_add(sumacc[:], sumacc[:], PT[:, di, :])

                for cb in range(NCCH):
                    gb = ci * NCCH + cb
                    # rowsum over partitions for these 128 queries
                    psr = ps_rs.tile([P, 1], f32, tag="rs")
                    nc.tensor.matmul(psr[:], lhsT=sumacc[:, ts(cb, P)],
                                     rhs=ones, start=True, stop=True)
                    stats = small.tile([P, 16], f32, tag="stats")
                    rs = stats[:, 0:1]
                    nc.vector.reciprocal(rs, psr[:])
                    # PV
                    po = ps_out.tile([P, L], f32, tag="out")
                    for di in range(NDB):
                        nc.tensor.matmul(po[:], lhsT=PT[:, di, ts(cb, P)],
                                         rhs=v_sb[:, di, :],
                                         start=(di == 0), stop=(di == NDB - 1))
                    out_sb = outp.tile([P, L], f32, tag="out")
                    nc.vector.tensor_scalar_mul(out_sb[:], po[:], rs)
                    nc.vector.tensor_add(out_sb[:], out_sb[:], apT[:, gb, :])
                    # LayerNorm over free dim (L)
                    st6 = stats[:, 2:8]
                    mv = stats[:, 8:10]
                    rstd = stats[:, 10:11]
                    nc.vector.bn_stats(st6, out_sb[:])
                    nc.vector.bn_aggr(mv, st6)
                    nc.scalar.activation(rstd, mv[:, 1:2], AF.Sqrt, bias=eps)
                    nc.vector.reciprocal(rstd, rstd)
                    nc.vector.tensor_scalar(out_sb[:], out_sb[:],
                                            mv[:, 0:1], rstd,
                                            ALU.subtract, ALU.mult)
                    if apply_gamma_beta:
                        nc.vector.tensor_mul(out_sb[:], out_sb[:],
                                             gb_pack[:, 0, :])
                        nc.vector.tensor_add(out_sb[:], out_sb[:],
                                             gb_pack[:, 1, :])
                    nc.sync.dma_start(out_d[bi, ds(gb * P, P), :], out_sb[:])

    nc.compile()
    return nc


def _get_nc(apply_qkv_bias, apply_gamma_beta, repeat=1):
    key = (apply_qkv_bias, apply_gamma_beta, repeat)
    if key not in _CACHE:
        _CACHE[key] = _build(*key)
    return _CACHE[key]


def _run(inputs, trace=False):
    from concourse import bass_utils

    a = np.ascontiguousarray(np.asarray(inputs["a"], dtype=np.float32))
    b = np.ascontiguousarray(np.asarray(inputs["b"], dtype=np.float32))
    get = lambda n: np.ascontiguousarray(np.asarray(inputs[n], dtype=np.float32))
    Wq, Wk, Wv = get("Wq"), get("Wk"), get("Wv")
    bq, bk, bv = get("bq"), get("bk"), get("bv")
    gamma, beta = get("gamma"), get("beta")

    apply_qkv_bias = bool(np.any(bq) or np.any(bk) or np.any(bv))
    apply_gamma_beta = bool(np.any(gamma != 1.0) or np.any(beta))
    nc = _get_nc(apply_qkv_bias, apply_gamma_beta)

    in_maps = []
    for c in range(NCORE):
        sl = slice(c * NB, (c + 1) * NB)
        in_maps.append({
            "a": np.ascontiguousarray(a[sl]), "b": np.ascontiguousarray(b[sl]),
            "Wq": Wq, "Wk": Wk, "Wv": Wv,
            "bq": bq, "bk": bk, "bv": bv,
            "gamma": gamma, "beta": beta,
        })
    res = bass_utils.run_bass_kernel_spmd(nc, in_maps,
                                          core_ids=list(range(NCORE)),
                                          trace=trace)
    out = np.concatenate(
        [res.results[c]["out"].reshape(NB, L, C) for c in range(NCORE)], axis=0)
    return out, res


def kernel(**inputs):
    out, _ = _run(inputs, trace=False)
    return out


# revision 23
# speedup vs baseline: 1.1225x; 1.0125x over previous
"""CrossDomainAttention TRN2 kernel: 8-core data-parallel over batch.

Reference computation (per batch element, a/b are (L, C) slices):
  ap = a.T (C, L);  q = ap@Wq.T+bq; k,v from b.T
  attn = softmax(q @ k.T / sqrt(L)) (C, C)
  out = LN(attn @ v + ap) over L, returned as the raw (C*L) buffer viewed (L, C)

Fast path (no qkv bias, no gamma/beta — the shipped inputs):
  scores = ap (Wq^T Wk) bp^T, so the q-projection disappears:
    GT = Wk^T Wq (precomputed once, fp8, x64)
    uT[l, d] = (GT^T b_raw)[l, d]  (fp8, keeps the x64)
    scoresT[d, c] = sum_l uT[l, d] a_raw[l, c]   (fp8 DoubleRow matmuls)
  PT[d, c] = exp(scoresT / (sqrt(L)*64)) stored fp8
  v8[d, m] = 16 * (b W_v^T)  fp8;  po = PT^T @ v8 (PSUM, fp8 DoubleRow)
  LayerNorm is scale-invariant per row, so skip the softmax division:
    x = po + (16*rowsum) * ap ;  out = (x - mean(x)) * rsqrt(var(x))
  rowsum via PE matmuls against a column of 16.0s.
"""

import numpy as np

B, L, C = 16, 512, 2048
NCORE = 8
NB = B // NCORE          # batch elements per core
P = 128
F = 512                  # matmul free-dim tile
NLC = L // P             # 4  l/m chunks
NDB = C // P             # 16 d-blocks / c-blocks
NCCH = C // F            # 4  c chunks
LN_EPS = 1e-5
SG = 64.0                # fp8 scale on GT
SV = 16.0                # fp8 scale on v (and the rowsum ones column)
ESC = 1.0 / (float(np.sqrt(L)) * SG)
INV_SQRT_L = 1.0 / float(np.sqrt(L))

_CACHE = {}


def _build_fast(repeat: int = 1):
    import concourse.bass as bass
    import concourse.tile as tile
    from concourse import bacc, mybir
    from concourse.bass import ts, ds
    from concourse.masks import make_identity
    from contextlib import ExitStack

    f32 = mybir.dt.float32
    f32r = mybir.dt.float32r
    f16 = mybir.dt.float16
    bf16 = mybir.dt.bfloat16
    f8 = mybir.dt.float8e4
    AF = mybir.ActivationFunctionType
    ALU = mybir.AluOpType
    DR = mybir.MatmulPerfMode.DoubleRow

    nc = bacc.Bacc("TRN2", target_bir_lowering=False, debug=False,
                   enable_asserts=False)

    a_d = nc.dram_tensor("a", (NB, L, C), f32, kind="ExternalInput").ap()
    b_d = nc.dram_tensor("b", (NB, L, C), f32, kind="ExternalInput").ap()
    w_d = {n: nc.dram_tensor(n, (L, L), f32, kind="ExternalInput").ap()
           for n in ("Wq", "Wk", "Wv")}
    for n in ("bq", "bk", "bv"):
        nc.dram_tensor(n, (L,), f32, kind="ExternalInput")
    nc.dram_tensor("gamma", (L,), f32, kind="ExternalInput")
    nc.dram_tensor("beta", (L,), f32, kind="ExternalInput")
    out_d = nc.dram_tensor("out", (NB, C, L), f32, kind="ExternalOutput").ap()

    with tile.TileContext(nc) as tc, ExitStack() as ctx:
        const = ctx.enter_context(tc.tile_pool(name="const", bufs=1))
        inp = ctx.enter_context(tc.tile_pool(name="inp", bufs=1))
        acts = ctx.enter_context(tc.tile_pool(name="acts", bufs=1))
        small = ctx.enter_context(tc.tile_pool(name="small", bufs=3))
        outp = ctx.enter_context(tc.tile_pool(name="outp", bufs=3))
        ps_mm = ctx.enter_context(tc.tile_pool(name="ps_mm", bufs=2, space="PSUM"))
        ps_out = ctx.enter_context(tc.tile_pool(name="ps_out", bufs=2, space="PSUM"))
        ps_tr = ctx.enter_context(tc.tile_pool(name="ps_tr", bufs=1, space="PSUM"))
        ps_rs = ctx.enter_context(tc.tile_pool(name="ps_rs", bufs=1, space="PSUM"))

        # ---- constants ----
        ident = const.tile([P, P], f32, tag="ident")
        make_identity(nc, ident)
        onesf = const.tile([P, 2], f32, tag="onesf")
        nc.vector.memset(onesf[:], SV)
        ones8 = const.tile([P, 2, 1], f8, tag="ones8")
        nc.vector.tensor_copy(ones8[:, :, 0], onesf[:])
        epsc = const.tile([P, 1], f32, tag="epsc")
        nc.vector.memset(epsc[:], LN_EPS)
        identb = const.tile([P, P], bf16, tag="identb")
        nc.vector.tensor_copy(identb[:], ident[:])

        # elem-0 input loads first: they gate the PE's first work
        pre_a = inp.tile([P, NLC, C], f32, tag="a_sb")
        pre_b = inp.tile([P, NLC, C], f32, tag="b_sb")
        for li in range(NLC):
            nc.sync.dma_start(pre_a[:, li, :], a_d[0, ds(li * P, P), :])
        wk_ld = inp.tile([P, NLC, L], f32, tag="wld")
        nc.sync.dma_start(wk_ld[:], w_d["Wk"].rearrange("(o p) l -> p o l", p=P))
        wq_ld = inp.tile([P, NLC, L], f32, tag="wld2")
        nc.sync.dma_start(wq_ld[:], w_d["Wq"].rearrange("(o p) l -> p o l", p=P))
        for li in range(NLC):
            nc.sync.dma_start(pre_b[:, li, :], b_d[0, ds(li * P, P), :])

        # elem-0 residual transposes are the PE's first possible work —
        # emit them before the weight matmuls (whose DMAs land later)
        apT0 = acts.tile([P, NDB, L], f16, tag="apT")
        for gb in range(NDB):
            # alternate PSUM banks so the PE never waits on the DVE drain
            if gb % 2 == 0:
                pst = ps_tr.tile([P, F], f32, tag="tr", name="pst")
            else:
                pst = ps_out.tile([P, L], f32, tag="out", name="pst")
            for li in range(NLC):
                nc.tensor.transpose(pst[:, ts(li, P)],
                                    pre_a[:, li, ts(gb, P)], ident[:])
            nc.vector.tensor_copy(apT0[:, gb, :], pst[:])

        # ---- weights ----
        # GT8 = SG * (Wk^T Wq): contraction over m using raw [m, l] layouts.
        wk_bf = inp.tile([P, NLC, L], bf16, tag="wbf")
        nc.vector.tensor_copy(wk_bf[:], wk_ld[:])
        wq_bf = inp.tile([P, NLC, L], bf16, tag="wbf2")
        nc.vector.tensor_copy(wq_bf[:], wq_ld[:])
        GT8 = const.tile([P, NLC, L], f8, tag="GT8")
        for lb in range(NLC):
            ps = ps_out.tile([P, F], f32, tag="out")
            for mi in range(NLC):
                nc.tensor.matmul(ps[:], lhsT=wk_bf[:, mi, ts(lb, P)],
                                 rhs=wq_bf[:, mi, :],
                                 start=(mi == 0), stop=(mi == NLC - 1))
            nc.scalar.mul(GT8[:, lb, :], ps[:], SG)
        # WvT8 = SV * Wv^T (PE transpose of raw Wv)
        wv_ld = inp.tile([P, NLC, L], f32, tag="wld")
        nc.sync.dma_start(wv_ld[:], w_d["Wv"].rearrange("(o p) l -> p o l", p=P))
        WvT8 = const.tile([P, NLC, L], f8, tag="WvT8")
        for li in range(NLC):
            pst = ps_tr.tile([P, F], f32, tag="tr")
            for mi in range(NLC):
                nc.tensor.transpose(pst[:, ts(mi, P)],
                                    wv_ld[:, mi, ts(li, P)], ident[:])
            nc.scalar.mul(WvT8[:, li, :], pst[:], SV)

        # ---- per batch element ----
        for idx, bi in enumerate([i % NB for i in range(NB * repeat)]):
            # loads + fp8 casts, spread across engines so the PE isn't
            # gated on one slow GpSimd cast chain at startup
            if idx == 0:
                a_sb, b_sb = pre_a, pre_b
                cast_eng = (nc.vector, nc.scalar, nc.vector, nc.scalar)
            else:
                a_sb = inp.tile([P, NLC, C], f32, tag="a_sb")
                b_sb = inp.tile([P, NLC, C], f32, tag="b_sb")
                cast_eng = (nc.vector, nc.scalar, nc.gpsimd, nc.gpsimd)
            a8 = acts.tile([P, NLC, C], f8, tag="a8", bufs=2)
            b8 = acts.tile([P, NLC, C], f8, tag="b8", bufs=2)

            def cast(eng, dst, src):
                if eng is nc.scalar:
                    eng.copy(dst, src)
                else:
                    eng.tensor_copy(dst, src)

            if idx == 0:
                for li in range(NLC):
                    cast(cast_eng[li], a8[:, li, :], a_sb[:, li, :])
                for li in range(NLC):
                    cast(cast_eng[li], b8[:, li, :], b_sb[:, li, :])
                apT = apT0
            else:
                # bf16 copy of a feeds half-cost PE transposes; its casts go
                # on DVE/ACT (GpSimd would gate the PE), while a8 — needed
                # latest, by the scores matmuls — moves wholly to GpSimd
                abf_eng = (nc.vector, nc.scalar, nc.vector, nc.scalar)
                a_bf = acts.tile([P, NLC, C], bf16, tag="a_bf")
                for li in range(NLC):
                    nc.sync.dma_start(a_sb[:, li, :], a_d[bi, ds(li * P, P), :])
                    cast(abf_eng[li], a_bf[:, li, :], a_sb[:, li, :])
                for li in range(NLC):
                    nc.sync.dma_start(b_sb[:, li, :], b_d[bi, ds(li * P, P), :])
                    cast(cast_eng[li], b8[:, li, :], b_sb[:, li, :])
                for li in range(NLC):
                    nc.gpsimd.tensor_copy(a8[:, li, :], a_sb[:, li, :])
                apT = acts.tile([P, NDB, L], f16, tag="apT")
                for gb in range(NDB):
                    if gb % 2 == 0:
                        pst = ps_tr.tile([P, F], f32, tag="tr", name="pst")
                    else:
                        pst = ps_out.tile([P, L], f32, tag="out", name="pst")
                    pbf = pst[:].bitcast(bf16)
                    for li in range(NLC):
                        nc.tensor.transpose(pbf[:, ts(li, P)],
                                            a_bf[:, li, ts(gb, P)], identb[:])
                    nc.vector.tensor_copy(apT[:, gb, :], pbf[:, 0:L])

            # uT[l_p, lb, d] = (GT^T b)[l, d], fp8 (keeps x SG)
            uT8 = acts.tile([P, NLC, C], f8, tag="uT8")
            for lb in range(NLC):
                for dc2 in range(0, NCCH, 2):
                    ps = ps_mm.tile([P, 2, F], f32, tag="mm")
                    for h in range(2):
                        for kp in range(0, NLC, 2):
                            nc.tensor.matmul(
                                ps[:, h, :],
                                lhsT=GT8[:, kp:kp + 2, ts(lb, P)],
                                rhs=b8[:, kp:kp + 2, ts(dc2 + h, F)],
                                start=(kp == 0), stop=(kp == NLC - 2),
                                perf_mode=DR)
                    nc.scalar.copy(uT8[:, lb, ds(dc2 * F, 2 * F)],
                                   ps.rearrange("p two f -> p (two f)"))

            # v8[d_p, di, m] = SV * (b Wv^T), fp8
            v8 = acts.tile([P, NDB, L], f8, tag="v8")
            for di2 in range(0, NDB, 2):
                ps = ps_mm.tile([P, 2, F], f32, tag="mm")
                for h in range(2):
                    for kp in range(0, NLC, 2):
                        nc.tensor.matmul(
                            ps[:, h, :],
                            lhsT=b8[:, kp:kp + 2, ts(di2 + h, P)],
                            rhs=WvT8[:, kp:kp + 2, :],
                            start=(kp == 0), stop=(kp == NLC - 2),
                            perf_mode=DR)
                nc.scalar.copy(
                    v8.rearrange("p n m -> p (n m)")[:, ds(di2 * F, 2 * F)],
                    ps.rearrange("p two f -> p (two f)"))

            # ---- attention, c-chunk at a time (PV/LN pipelined one behind) ----
            def emit_scores(ci):
                PT8 = acts.tile([P, NDB, F], f8, tag="PT", bufs=2)
                psr4 = small.tile([P, NCCH], f32, tag="psr", bufs=2)
                for di2 in range(0, NDB, 2):
                    ps = ps_mm.tile([P, 2, F], f32, tag="mm")
                    for h in range(2):
                        for kp in range(0, NLC, 2):
                            nc.tensor.matmul(
                                ps[:, h, :],
                                lhsT=uT8[:, kp:kp + 2, ts(di2 + h, P)],
                                rhs=a8[:, kp:kp + 2, ts(ci, F)],
                                start=(kp == 0), stop=(kp == NLC - 2),
                                perf_mode=DR)
                    nc.scalar.activation(
                        PT8.rearrange("p n f -> p (n f)")[:, ds(di2 * F, 2 * F)],
                        ps.rearrange("p two f -> p (two f)"), AF.Exp, scale=ESC)
                # psr[c] = SV * rowsum, per 128-query block
                for cb in range(NCCH):
                    psp = ps_rs.tile([P, 1], f32, tag="rs")
                    for di2 in range(0, NDB, 2):
                        nc.tensor.matmul(psp[:],
                                         lhsT=PT8[:, di2:di2 + 2, ts(cb, P)],
                                         rhs=ones8[:],
                                         start=(di2 == 0), stop=(di2 == NDB - 2),
                                         perf_mode=DR)
                    nc.vector.tensor_copy(psr4[:, cb:cb + 1], psp[:])
                return PT8, psr4

            def emit_pv_ln(ci, PT8, psr4, tail=False):
                mv4 = small.tile([P, NCCH, 2], f32, tag="mv4", bufs=2)
                xs = []
                for cb in range(NCCH):
                    po = ps_out.tile([P, L], f32, tag="out")
                    for di2 in range(0, NDB, 2):
                        nc.tensor.matmul(po[:],
                                         lhsT=PT8[:, di2:di2 + 2, ts(cb, P)],
                                         rhs=v8[:, di2:di2 + 2, :],
                                         start=(di2 == 0), stop=(di2 == NDB - 2),
                                         perf_mode=DR)
                    x_sb = outp.tile([P, L], f32, tag="x", bufs=6)
                    nc.vector.affine_then_add(x_sb[:], in0=apT[:, ci * NCCH + cb, :],
                                              in1=po[:], scale=psr4[:, cb:cb + 1],
                                              bias=0.0)
                    st6 = small.tile([P, 6], f32, tag="st6")
                    nc.vector.bn_stats(st6[:], x_sb[:])
                    nc.vector.bn_aggr(mv4[:, cb, :], st6[:])
                    xs.append(x_sb)
                    if tail:
                        # per-cb finish: shortest serial chain at kernel end
                        sdc = small.tile([P, 2], f32, tag="sdc")
                        nc.scalar.activation(sdc[:, 0:1], mv4[:, cb, 1:2],
                                             AF.Sqrt, bias=epsc[:])
                        nc.vector.reciprocal(sdc[:, 1:2], sdc[:, 0:1])
                        out_sb = outp.tile([P, L], f32, tag="osb", bufs=3)
                        nc.vector.tensor_scalar(out_sb[:], x_sb[:],
                                                mv4[:, cb, 0:1], sdc[:, 1:2],
                                                ALU.subtract, ALU.mult)
                        nc.sync.dma_start(out_d[bi, ds((ci * NCCH + cb) * P, P), :],
                                          out_sb[:])
                if tail:
                    return
                sd4 = small.tile([P, 2 * NCCH], f32, tag="sd4")
                nc.scalar.activation(sd4[:, 0:NCCH], mv4[:, :, 1], AF.Sqrt,
                                     bias=epsc[:])
                nc.vector.reciprocal(sd4[:, NCCH:2 * NCCH], sd4[:, 0:NCCH])
                for cb in range(NCCH):
                    gb = ci * NCCH + cb
                    out_sb = outp.tile([P, L], f32, tag="osb", bufs=3)
                    nc.vector.tensor_scalar(out_sb[:], xs[cb][:],
                                            mv4[:, cb, 0:1],
                                            sd4[:, NCCH + cb:NCCH + cb + 1],
                                            ALU.subtract, ALU.mult)
                    nc.sync.dma_start(out_d[bi, ds(gb * P, P), :], out_sb[:])

            prev = None
            for ci in range(NCCH):
                cur = emit_scores(ci)
                if prev is not None:
                    emit_pv_ln(ci - 1, *prev)
                prev = cur
            emit_pv_ln(NCCH - 1, *prev, tail=(idx == NB * repeat - 1))

    nc.compile()
    return nc


def _build(apply_qkv_bias: bool, apply_gamma_beta: bool, repeat: int = 1):
    if not apply_qkv_bias and not apply_gamma_beta:
        return _build_fast(repeat)
    return _build_generic(apply_qkv_bias, apply_gamma_beta, repeat)


def _build_generic(apply_qkv_bias: bool, apply_gamma_beta: bool, repeat: int = 1):
    import concourse.bass as bass
    import concourse.tile as tile
    from concourse import bacc, mybir
    from concourse.bass import ts, ds
    from concourse.masks import make_identity
    from contextlib import ExitStack

    f32 = mybir.dt.float32
    f16 = mybir.dt.float16
    bf16 = mybir.dt.bfloat16
    AF = mybir.ActivationFunctionType
    ALU = mybir.AluOpType

    nc = bacc.Bacc("TRN2", target_bir_lowering=False, debug=False,
                   enable_asserts=False)

    a_d = nc.dram_tensor("a", (NB, L, C), f32, kind="ExternalInput").ap()
    b_d = nc.dram_tensor("b", (NB, L, C), f32, kind="ExternalInput").ap()
    w_d = {n: nc.dram_tensor(n, (L, L), f32, kind="ExternalInput").ap()
           for n in ("Wq", "Wk", "Wv")}
    bias_d = {n: nc.dram_tensor(n, (L,), f32, kind="ExternalInput").ap()
              for n in ("bq", "bk", "bv")}
    gamma_d = nc.dram_tensor("gamma", (L,), f32, kind="ExternalInput").ap()
    beta_d = nc.dram_tensor("beta", (L,), f32, kind="ExternalInput").ap()
    out_d = nc.dram_tensor("out", (NB, C, L), f32, kind="ExternalOutput").ap()

    def bcast_p(ap1d):
        # broadcast a 1-D DRAM AP across all 128 partitions (DMA source)
        return bass.AP(tensor=ap1d.tensor, offset=ap1d.offset,
                       ap=[[0, P]] + [list(d) for d in ap1d.ap])

    with tile.TileContext(nc) as tc, ExitStack() as ctx:
        const = ctx.enter_context(tc.tile_pool(name="const", bufs=1))
        inp = ctx.enter_context(tc.tile_pool(name="inp", bufs=1))
        acts = ctx.enter_context(tc.tile_pool(name="acts", bufs=1))
        small = ctx.enter_context(tc.tile_pool(name="small", bufs=3))
        outp = ctx.enter_context(tc.tile_pool(name="outp", bufs=3))
        ps_mm = ctx.enter_context(tc.tile_pool(name="ps_mm", bufs=4, space="PSUM"))
        ps_out = ctx.enter_context(tc.tile_pool(name="ps_out", bufs=2, space="PSUM"))
        ps_tr = ctx.enter_context(tc.tile_pool(name="ps_tr", bufs=1, space="PSUM"))
        ps_rs = ctx.enter_context(tc.tile_pool(name="ps_rs", bufs=1, space="PSUM"))

        # ---- constants ----
        ident = const.tile([P, P], f32, tag="ident")
        make_identity(nc, ident)
        cpack = const.tile([P, 16], f32, tag="cpack")
        nc.vector.memset(cpack[:, 0:1], 1.0)      # ones column (reduce rhs)
        nc.vector.memset(cpack[:, 1:2], LN_EPS)
        ones = cpack[:, 0:1]
        eps = cpack[:, 1:2]
        bias_col = {}
        bv_bc = None
        if apply_qkv_bias:
            for i, n in enumerate(("bq", "bk")):
                dst = cpack[:, 2 + 4 * i: 2 + 4 * (i + 1)]
                nc.sync.dma_start(dst, bias_d[n].rearrange("(o p) -> p o", p=P))
                bias_col[n] = dst
            bv_bc = const.tile([P, L], f32, tag="bv_bc")
            nc.sync.dma_start(bv_bc[:], bcast_p(bias_d["bv"]))
        if apply_gamma_beta:
            gb_pack = const.tile([P, 2, L], f32, tag="gb")
            nc.sync.dma_start(gb_pack[:, 0, :], bcast_p(gamma_d))
            nc.sync.dma_start(gb_pack[:, 1, :], bcast_p(beta_d))

        # ---- weights: load W[m, l] fp32, transpose -> WT[l_p, li, m] bf16 ----
        WT = {}
        for n in ("Wq", "Wk", "Wv"):
            wld = inp.tile([P, NLC, F], f32, tag="bh")
            nc.sync.dma_start(wld[:], w_d[n].rearrange("(o p) l -> p o l", p=P))
            wt = const.tile([P, NLC, L], bf16, tag=f"WT_{n}")
            for mi in range(NLC):
                pst = ps_tr.tile([P, F], f32, tag="tr")
                for li in range(NLC):
                    nc.tensor.transpose(pst[:, ts(li, P)],
                                        wld[:, mi, ts(li, P)], ident[:])
                nc.vector.tensor_copy(
                    wt[:, :, ts(mi, P)],
                    pst.rearrange("p (li f) -> p li f", f=P))
            WT[n] = wt

        # ---- per batch element ----
        for bi in [i % NB for i in range(NB * repeat)]:
            # a: load fp32 per l-chunk, cast to bf16 (GPSIMD)
            a_sb = inp.tile([P, NLC, C], f32, tag="a")
            a_bf = acts.tile([P, NLC, C], bf16, tag="a_bf")
            for li in range(NLC):
                nc.sync.dma_start(a_sb[:, li, :],
                                  a_d[bi, ds(li * P, P), :])
                nc.gpsimd.tensor_copy(a_bf[:, li, :], a_sb[:, li, :])
            apT = acts.tile([P, NDB, L], f16, tag="apT")

            def emit_apt(gb, a_sb=a_sb, apT=apT):
                pst = ps_tr.tile([P, F], f32, tag="tr", name="pst")
                for li in range(NLC):
                    nc.tensor.transpose(pst[:, ts(li, P)],
                                        a_sb[:, li, ts(gb, P)], ident[:])
                nc.vector.tensor_copy(apT[:, gb, :], pst[:])

            # a few up-front (they only need a_sb); the rest interleave into
            # the QKV loop as PE filler between PSUM-limited matmul groups
            apt_queue = list(range(NDB))
            for _ in range(4):
                emit_apt(apt_queue.pop(0))

            # b: load fp32 in halves, cast to bf16
            b_bf = acts.tile([P, NLC, C], bf16, tag="bpt", bufs=2)
            for h in range(2):
                b_sb = inp.tile([P, 2, C], f32, tag="bh")
                nc.sync.dma_start(
                    b_sb[:],
                    b_d[bi, ds(h * 2 * P, 2 * P), :].rearrange(
                        "(o p) c -> p o c", p=P))
                for li in range(2):
                    nc.gpsimd.tensor_copy(b_bf[:, h * 2 + li, :], b_sb[:, li, :])

            # qT[m_p, mi, c], kT[m_p, mi, c] (bf16)
            qT = acts.tile([P, NLC, C], bf16, tag="qT")
            kT = acts.tile([P, NLC, C], bf16, tag="kT")
            for wname, bname, src, dst in (("Wq", "bq", a_bf, qT),
                                           ("Wk", "bk", b_bf, kT)):
                for mi in range(NLC):
                    for ci in range(NCCH):
                        ps = ps_mm.tile([P, F], f32, tag="mm")
                        for li in range(NLC):
                            nc.tensor.matmul(ps[:],
                                             lhsT=WT[wname][:, li, ts(mi, P)],
                                             rhs=src[:, li, ts(ci, F)],
                                             start=(li == 0), stop=(li == NLC - 1))
                        dslice = dst[:, mi, ts(ci, F)]
                        if apply_qkv_bias:
                            nc.scalar.activation(dslice, ps[:], AF.Identity,
                                                 bias=bias_col[bname][:, mi:mi + 1])
                        elif (mi + ci) % 2 == 0:
                            nc.scalar.copy(dslice, ps[:])
                        else:
                            nc.vector.tensor_copy(dslice, ps[:])
                        if apt_queue:
                            emit_apt(apt_queue.pop(0))

            # v[d_p, di, m] (bf16)
            v_sb = acts.tile([P, NDB, L], bf16, tag="v")
            for di in range(NDB):
                ps = ps_mm.tile([P, F], f32, tag="mm")
                for li in range(NLC):
                    nc.tensor.matmul(ps[:], lhsT=b_bf[:, li, ts(di, P)],
                                     rhs=WT["Wv"][:, li, :],
                                     start=(li == 0), stop=(li == NLC - 1))
                if di % 2 == 0:
                    nc.scalar.copy(v_sb[:, di, :], ps[:])
                else:
                    nc.vector.tensor_copy(v_sb[:, di, :], ps[:])
                if apply_qkv_bias:
                    nc.vector.tensor_add(v_sb[:, di, :], v_sb[:, di, :],
                                         bv_bc[:, :])

            # ---- attention, c-chunk at a time ----
            for ci in range(NCCH):
                PT = acts.tile([P, NDB, F], bf16, tag="bpt", bufs=2)
                sumacc = small.tile([P, F], f32, tag="sumacc")
                for di in range(NDB):
                    ps = ps_mm.tile([P, F], f32, tag="mm")
                    for mi in range(NLC):
                        nc.tensor.matmul(ps[:], lhsT=kT[:, mi, ts(di, P)],
                                         rhs=qT[:, mi, ts(ci, F)],
                                         start=(mi == 0), stop=(mi == NLC - 1))
                    nc.scalar.activation(PT[:, di, :], ps[:], AF.Exp,
                                         scale=INV_SQRT_L)
                    if di == 0:
                        nc.vector.tensor_copy(sumacc[:], PT[:, di, :])
                    else:
                        nc.vector.tensor_add(sumacc[:], sumacc[:], PT[:, di, :])

                for cb in range(NCCH):
                    gb = ci * NCCH + cb
                    # rowsum over partitions for these 128 queries
                    psr = ps_rs.tile([P, 1], f32, tag="rs")
                    nc.tensor.matmul(psr[:], lhsT=sumacc[:, ts(cb, P)],
                                     rhs=ones, start=True, stop=True)
                    stats = small.tile([P, 16], f32, tag="stats")
                    rs = stats[:, 0:1]
                    nc.vector.reciprocal(rs, psr[:])
                    # PV
                    po = ps_out.tile([P, L], f32, tag="out")
                    for di in range(NDB):
                        nc.tensor.matmul(po[:], lhsT=PT[:, di, ts(cb, P)],
                                         rhs=v_sb[:, di, :],
                                         start=(di == 0), stop=(di == NDB - 1))
                    out_sb = outp.tile([P, L], f32, tag="out")
                    nc.vector.tensor_scalar_mul(out_sb[:], po[:], rs)
                    nc.vector.tensor_add(out_sb[:], out_sb[:], apT[:, gb, :])
                    # LayerNorm over free dim (L)
                    st6 = stats[:, 2:8]
                    mv = stats[:, 8:10]
                    rstd = stats[:, 10:11]
                    nc.vector.bn_stats(st6, out_sb[:])
                    nc.vector.bn_aggr(mv, st6)
                    nc.scalar.activation(rstd, mv[:, 1:2], AF.Sqrt, bias=eps)
                    nc.vector.reciprocal(rstd, rstd)
                    nc.vector.tensor_scalar(out_sb[:], out_sb[:],
                                            mv[:, 0:1], rstd,
                                            ALU.subtract, ALU.mult)
                    if apply_gamma_beta:
                        nc.vector.tensor_mul(out_sb[:], out_sb[:],
                                             gb_pack[:, 0, :])
                        nc.vector.tensor_add(out_sb[:], out_sb[:],
                                             gb_pack[:, 1, :])
                    nc.sync.dma_start(out_d[bi, ds(gb * P, P), :], out_sb[:])

    nc.compile()
    return nc


def _get_nc(apply_qkv_bias, apply_gamma_beta, repeat=1):
    key = (apply_qkv_bias, apply_gamma_beta, repeat)
    if key not in _CACHE:
        _CACHE[key] = _build(*key)
    return _CACHE[key]


def _run(inputs, trace=False):
    from concourse import bass_utils

    a = np.ascontiguousarray(np.asarray(inputs["a"], dtype=np.float32))
    b = np.ascontiguousarray(np.asarray(inputs["b"], dtype=np.float32))
    get = lambda n: np.ascontiguousarray(np.asarray(inputs[n], dtype=np.float32))
    Wq, Wk, Wv = get("Wq"), get("Wk"), get("Wv")
    bq, bk, bv = get("bq"), get("bk"), get("bv")
    gamma, beta = get("gamma"), get("beta")

    apply_qkv_bias = bool(np.any(bq) or np.any(bk) or np.any(bv))
    apply_gamma_beta = bool(np.any(gamma != 1.0) or np.any(beta))
    nc = _get_nc(apply_qkv_bias, apply_gamma_beta)

    in_maps = []
    for c in range(NCORE):
        sl = slice(c * NB, (c + 1) * NB)
        in_maps.append({
            "a": np.ascontiguousarray(a[sl]), "b": np.ascontiguousarray(b[sl]),
            "Wq": Wq, "Wk": Wk, "Wv": Wv,
            "bq": bq, "bk": bk, "bv": bv,
            "gamma": gamma, "beta": beta,
        })
    res = bass_utils.run_bass_kernel_spmd(nc, in_maps,
                                          core_ids=list(range(NCORE)),
                                          trace=trace)
    out = np.concatenate(
        [res.results[c]["out"].reshape(NB, L, C) for c in range(NCORE)], axis=0)
    return out, res


def kernel(**inputs):
    out, _ = _run(inputs, trace=False)
    return out
